# revision 97
# baseline (speedup 1.0000x reference)
"""GAT classifier on 8 trn2 NeuronCores (Bass/Tile) — v2.

Sharding: 1D node partition (6250 nodes/core); edges assigned to the core
owning their dst node, sorted by dst into 128-node chunks.

v2 design (vs v1): the per-edge SWDGE gather count is halved and payloads
move to bf16. Per edge per layer there is exactly ONE gathered element:
  L1: 512B bf16 row [h0(64)|1|h1(64)|1|as0|as1|pad]
  L2: 256B bf16 row [h(64)|1|as|pad]
The dst-side attention term ad_e is produced without any gather: a host
-streamed fp8 one-hot S0T[j,e] (slot-of-edge) is matmul'd against the
local per-chunk ad table (PE, 2 cols), giving per-edge ad in PSUM.
Coefficients are computed directly: coef = exp(max(s, 0.2*s)), s = as+ad.
The segmented softmax + aggregation stays as PSUM-accumulated selection
matmuls with bf16 S matrices; denominators ride the baked-in "1" columns.
Layer tables are AllGathered between layers; pooling uses an AllReduce.
"""
import math
import os
import sys
from contextlib import ExitStack
from dataclasses import dataclass

import numpy as np

for _p in ("/opt/trn_rl_repo", "/root/.axon_site/_ro/trn_rl_repo"):
    if os.path.isdir(_p) and _p not in sys.path:
        sys.path.insert(0, _p)

import concourse.bacc as bacc
import concourse.bass as bass
import concourse.mybir as mybir
import concourse.tile as tile
from concourse.tile import add_dep_helper
from concourse.bass_utils import run_bass_kernel_spmd
from concourse.masks import make_identity

P = 128
AF = mybir.ActivationFunctionType
ALU = mybir.AluOpType
F32 = mybir.dt.float32
BF16 = mybir.dt.bfloat16
FP8 = mybir.dt.float8e4
I16 = mybir.dt.int16
NP_BF16 = mybir.dt.np(BF16)
NP_FP8 = mybir.dt.np(FP8)

ROW1 = 256   # L1 table row: fp8 slots [h0|1|h1|1|pad|as0,as1(bf16@66,67)] -> 256B
ROW2 = 128   # L2 table row: bf16 slots [h|1|as|pad] -> 256B


@dataclass
class Cfg:
    N: int = 50000
    E0: int = 800000
    IN: int = 128
    HID: int = 64
    G: int = 512
    CORES: int = 8
    NPC: int = 0
    CH: int = 0
    HALF: int = 0
    NVA: int = 0
    G_CH: int = 1
    lay1: dict = None  # layer-1 gather layout (natural, split@32768)
    lay2: dict = None  # layer-2 gather layout (block, split@NVA)

    @property
    def NCH(self):  # padded per-core node count
        return self.CH * P


def plan_cfg(N, E0, G, CORES=8):
    c = Cfg(N=N, E0=E0, G=G, CORES=CORES)
    c.G_CH = int(os.environ.get("G_CH", "1"))
    assert N % CORES == 0
    c.NPC = N // CORES
    c.CH = math.ceil(c.NPC / P)
    # Layer 1 uses the natural core-major table layout (ONE AllGather,
    # address space split at 32768 for int16 indices).  Layer 2 uses a
    # block layout (block A = every core's first NVA rows) so its table
    # AllGather can be split in two, with the A-half overlapping layer 1's
    # edge phase.
    c.HALF = 1 << 15
    assert (N - c.HALF) <= 32767
    # NVA=4096 gives layer 2's block layout the same 65/35 lo/hi split as
    # layer 1's natural layout (same padding), with block-A indices still
    # fitting int16 (max CORES*NVA-1 = 32767).
    c.NVA = 32 * P
    assert 32 * P < c.NPC and 32 <= c.CH
    assert CORES * c.NVA <= 32768 and CORES * (c.NPC - c.NVA) <= 32767
    return c


# ----------------------------------------------------------------- host prep

def prep_edges(cfg, src, dst, half_map, sfx):
    """Per-core edge arrays with per-chunk section sizes (max across cores,
    so the SPMD program shape is identical). Returns (layout, per-core dicts).
    half_map(s) -> (half, idx) arrays for the given table layout."""
    owner = dst // cfg.NPC
    per_core = []
    cnts = np.zeros((cfg.CORES, cfg.CH, 2), np.int64)
    for c in range(cfg.CORES):
        m = owner == c
        s0 = src[m]
        dl = dst[m] - c * cfg.NPC
        chunk = dl >> 7
        half, s = half_map(s0)
        order = np.lexsort((s, half, chunk))
        s, dl, chunk, half = s[order], dl[order], chunk[order], half[order]
        key = chunk * 2 + half
        cnt = np.bincount(key, minlength=cfg.CH * 2).reshape(cfg.CH, 2)
        cnts[c] = cnt
        per_core.append((s, dl, cnt))
    cmax = cnts.max(axis=0)  # [CH, 2]
    TL = np.maximum((cmax[:, 0] + 127) // P, 1)
    TH = np.maximum((cmax[:, 1] + 127) // P, 1)
    lo_sz, hi_sz = TL * P, TH * P
    lay = dict(
        TL=tuple(int(v) for v in TL), TH=tuple(int(v) for v in TH),
        LO_OFF=tuple(int(v) for v in np.concatenate([[0], np.cumsum(lo_sz)])),
        HI_OFF=tuple(int(v) for v in np.concatenate([[0], np.cumsum(hi_sz)])),
        S_OFF=tuple(int(v) for v in np.concatenate([[0], np.cumsum(lo_sz + hi_sz)])),
        TOT_LO=int(lo_sz.sum()), TOT_HI=int(hi_sz.sum()))
    lay["TOT_E"] = lay["TOT_LO"] + lay["TOT_HI"]

    out = []
    for c in range(cfg.CORES):
        s, dl, cnt = per_core[c]
        gl = np.zeros(lay["TOT_LO"], np.int16)
        gh = np.zeros(lay["TOT_HI"], np.int16)
        sl = np.full(lay["TOT_E"], 300.0, np.float32)
        ofs = np.zeros(cfg.CH * 2 + 1, np.int64)
        np.cumsum(cnt.reshape(-1), out=ofs[1:])
        for k in range(cfg.CH):
            nlo, nhi = int(cnt[k, 0]), int(cnt[k, 1])
            a = ofs[2 * k]
            gl[lay["LO_OFF"][k]:lay["LO_OFF"][k] + nlo] = s[a:a + nlo]
            sl[lay["S_OFF"][k]:lay["S_OFF"][k] + nlo] = (dl[a:a + nlo] & 127).astype(np.float32)
            b = ofs[2 * k + 1]
            gh[lay["HI_OFF"][k]:lay["HI_OFF"][k] + nhi] = s[b:b + nhi]
            shi = lay["S_OFF"][k] + lo_sz[k]
            sl[shi:shi + nhi] = (dl[b:b + nhi] & 127).astype(np.float32)

        def wrap16(a):  # idx i -> [i % 16, i // 16], replicated over 8 groups
            w = a.reshape(-1, 16).T.copy()
            return np.tile(w, (8, 1)).astype(np.int16)

        # S0T fp8 one-hot blocks: [128 j, TOT_E] — col S_OFF[c]+t*128+e is
        # one at row slot_e (pad slots 300 -> all-zero column).
        sl_t = sl.reshape(-1, P)  # [tile, e] slot values
        idx = sl_t.astype(np.int32)
        ntile = sl_t.shape[0]
        s0t = np.zeros((ntile, P, P), NP_FP8)  # [tile, e, j]
        tt, ee = np.nonzero(idx < P)
        s0t[tt, ee, idx[tt, ee]] = 1.0
        s0e = np.ascontiguousarray(s0t.transpose(1, 0, 2).reshape(P, ntile * P))
        s0t = np.ascontiguousarray(s0t.transpose(2, 0, 1).reshape(P, ntile * P))

        out.append({f"gl{sfx}": wrap16(gl), f"gh{sfx}": wrap16(gh),
                    f"s0t{sfx}": s0t, f"s0e{sfx}": s0e})
    return lay, out


def balance_perm(cfg, dst):
    """Per-core node->slot permutation equalizing per-chunk edge counts.
    Returns perm[global] -> new global id (same core, reassigned chunk)."""
    N, CORES, NPC, CH = cfg.N, cfg.CORES, cfg.NPC, cfg.CH
    deg = np.bincount(dst, minlength=N).astype(np.int64)
    perm = np.empty(N, np.int64)
    for c in range(CORES):
        d = deg[c * NPC:(c + 1) * NPC]
        order = np.argsort(-d, kind="stable")
        loads = np.zeros(CH, np.int64)
        counts = np.zeros(CH, np.int64)
        cap = np.full(CH, P, np.int64)
        cap[CH - 1] = NPC - (CH - 1) * P if NPC % P else P
        newloc = np.empty(NPC, np.int64)
        import heapq
        heap = [(0, k) for k in range(CH)]
        heapq.heapify(heap)
        for i in order:
            while True:
                l, k = heapq.heappop(heap)
                if counts[k] < cap[k]:
                    break
            newloc[i] = k * P + counts[k]
            counts[k] += 1
            loads[k] += d[i]
            if counts[k] < cap[k]:
                heapq.heappush(heap, (loads[k], k))
        perm[c * NPC:(c + 1) * NPC] = c * NPC + newloc
    return perm


def prep_inputs(cfg, x, edge_index, batch, W1, a_src1, a_dst1, W2, a_src2, a_dst2, fcW):
    # self-loops are NOT materialized as edges; their contribution is added
    # locally per chunk (coef_self * own row) in the fin callbacks.
    N, CORES, NPC, CH = cfg.N, cfg.CORES, cfg.NPC, cfg.CH
    src = edge_index[0].astype(np.int64)
    dst = edge_index[1].astype(np.int64)
    perm = balance_perm(cfg, dst)
    src, dst = perm[src], perm[dst]
    inv = np.empty(N, np.int64)
    inv[perm] = np.arange(N)
    x = x[inv]
    batch = batch[inv]

    def half_nat(s):  # layer-1: natural layout split at 32768
        half = (s >= cfg.HALF).astype(np.int64)
        return half, np.where(half == 0, s, s - cfg.HALF)

    def half_blk(s):  # layer-2: block layout split at NVA per core
        s_c, s_l = s // NPC, s % NPC
        half = (s_l >= cfg.NVA).astype(np.int64)
        return half, np.where(half == 0, s_c * cfg.NVA + s_l,
                              s_c * (NPC - cfg.NVA) + (s_l - cfg.NVA))

    cfg.lay1, edges1 = prep_edges(cfg, src, dst, half_nat, "1")
    cfg.lay2, edges2 = prep_edges(cfg, src, dst, half_blk, "2")
    edges = [e1 | e2 for e1, e2 in zip(edges1, edges2)]

    H = 2
    HID = cfg.HID
    # rhs1 cols: [W_h0(64) | 0 | W_h1(64) | 0 | as0 as1 ad0 ad1 | pad] = 136;
    # the zero gap columns become the denominator "1" slots (memset on-chip),
    # so the MM output is the table row layout directly.
    rhs1 = np.zeros((cfg.IN, 136), np.float32)
    rhs1[:, 0:64] = W1[:, 0:HID]
    rhs1[:, 65:129] = W1[:, HID:2 * HID]
    for h in range(H):
        rhs1[:, 130 + h] = W1[:, h * HID:(h + 1) * HID] @ a_src1[h]
        rhs1[:, 132 + h] = W1[:, h * HID:(h + 1) * HID] @ a_dst1[h]
    # rhs2 cols: [W2(64) | 0 | as2 ad2 | pad] = 68
    rhs2 = np.zeros((H * HID, 68), np.float32)
    rhs2[:, 0:HID] = W2
    rhs2[:, HID + 1] = W2 @ a_src2[0]
    rhs2[:, HID + 2] = W2 @ a_dst2[0]

    iota512 = np.tile(np.arange(cfg.G, dtype=np.float32), (P, 1))
    cnt = np.bincount(batch, minlength=cfg.G).astype(np.float32)
    invc = 1.0 / np.maximum(cnt, 1.0)
    invc_b = np.tile(invc, (HID, 1)).astype(np.float32)

    xT = np.zeros((cfg.IN, CORES * cfg.NCH), NP_BF16)
    gsl = np.full((CORES, cfg.NCH), 999.0, np.float32)
    for c in range(CORES):
        xT[:, c * cfg.NCH:c * cfg.NCH + NPC] = x[c * NPC:(c + 1) * NPC].T
        gsl[c, :NPC] = batch[c * NPC:(c + 1) * NPC]

    in_maps = []
    for c in range(CORES):
        in_maps.append(dict(
            xT=np.ascontiguousarray(xT[:, c * cfg.NCH:(c + 1) * cfg.NCH]),
            rhs1=rhs1.astype(NP_BF16), rhs2=rhs2.astype(NP_BF16),
            fcW=fcW.astype(np.float32),
            iota512=iota512, invc=invc_b,
            gslot=gsl[c].reshape(CH, P).T.copy(),
            **edges[c],
        ))
    return in_maps


# -------------------------------------------------------------- bass builder

def build_nc(cfg):
    N, CH, NPC = cfg.N, cfg.CH, cfg.NPC
    HID, G = cfg.HID, cfg.G
    HALF, NVA = cfg.HALF, cfg.NVA
    CH_A = NVA // P
    BHALF = cfg.CORES * NVA
    R = list(range(cfg.CORES))

    NQ = int(os.environ.get("GATHER_QUEUES", "4"))
    nc = bacc.Bacc(num_swdge_queues=NQ)
    pi = lambda n, s, d=F32: nc.declare_dram_parameter(n, s, d, isOutput=False)
    xT = pi("xT", [cfg.IN, cfg.NCH], BF16)
    rhs1 = pi("rhs1", [cfg.IN, 136], BF16)
    rhs2 = pi("rhs2", [2 * HID, 68], BF16)
    fcW = pi("fcW", [HID, 2])
    iota512 = pi("iota512", [P, G])
    invc = pi("invc", [HID, G])
    gslot = pi("gslot", [P, CH])
    gl1 = pi("gl1", [P, cfg.lay1["TOT_LO"] // 16], I16)
    gh1 = pi("gh1", [P, cfg.lay1["TOT_HI"] // 16], I16)
    s0t1 = pi("s0t1", [P, cfg.lay1["TOT_E"]], FP8)
    s0e1 = pi("s0e1", [P, cfg.lay1["TOT_E"]], FP8)
    gl2 = pi("gl2", [P, cfg.lay2["TOT_LO"] // 16], I16)
    gh2 = pi("gh2", [P, cfg.lay2["TOT_HI"] // 16], I16)
    s0t2 = pi("s0t2", [P, cfg.lay2["TOT_E"]], FP8)
    s0e2 = pi("s0e2", [P, cfg.lay2["TOT_E"]], FP8)
    out_lg = nc.declare_dram_parameter("out_lg", [G, 2], F32, isOutput=True)

    shard1 = nc.dram_tensor("shard1", [NPC, ROW1], FP8)
    table1 = nc.dram_tensor("table1", [N, ROW1], FP8, addr_space="Shared")
    shard2 = nc.dram_tensor("shard2", [NPC, ROW2], BF16)
    table2 = nc.dram_tensor("table2", [N, ROW2], BF16, addr_space="Shared")
    pool_loc = nc.dram_tensor("pool_loc", [G, 2], F32)
    pool_sh = nc.dram_tensor("pool_sh", [G, 2], F32, addr_space="Shared")

    # Gathers round-robin across SWDGE queues; each queue runs on its own Q7
    # core pair (queue q -> cores 2q,2q+1) and the ucode's own ring
    # await_space provides flow control, so no software pacing is needed.
    gather_rr = [0]

    def paced_gather(probe_pool, **kw):
        q = gather_rr[0] % NQ
        gather_rr[0] += 1
        return nc.gpsimd.dma_gather(single_packet=False, queue_num=q, **kw)

    with tile.TileContext(nc) as tc, ExitStack() as ctx:
        cp = ctx.enter_context(tc.tile_pool(name="const", bufs=1))
        dio = ctx.enter_context(tc.tile_pool(name="dio", bufs=3))
        dps = ctx.enter_context(tc.tile_pool(name="dps", bufs=2, space="PSUM"))
        o1p = ctx.enter_context(tc.tile_pool(name="o1p", bufs=1))
        ixp = ctx.enter_context(tc.tile_pool(name="ixp", bufs=10))
        gp = ctx.enter_context(tc.tile_pool(name="gp", bufs=12))
        stp = ctx.enter_context(tc.tile_pool(name="stp", bufs=6))
        sxp = ctx.enter_context(tc.tile_pool(name="sxp", bufs=8))
        xp = ctx.enter_context(tc.tile_pool(name="xp", bufs=8))
        ups = ctx.enter_context(tc.tile_pool(name="ups", bufs=3, space="PSUM"))
        aps = ctx.enter_context(tc.tile_pool(name="aps", bufs=2, space="PSUM"))
        pps = ctx.enter_context(tc.tile_pool(name="pps", bufs=1, space="PSUM"))
        fin = ctx.enter_context(tc.tile_pool(name="fin", bufs=3))

        io512 = cp.tile([P, G], F32)
        nc.sync.dma_start(out=io512[:], in_=iota512[:])
        r1sb = cp.tile([cfg.IN, 136], BF16)
        nc.sync.dma_start(out=r1sb[:], in_=rhs1[:])
        r2sb = cp.tile([2 * HID, 68], BF16)
        nc.sync.dma_start(out=r2sb[:], in_=rhs2[:])
        fcsb = cp.tile([HID, 2], F32)
        nc.sync.dma_start(out=fcsb[:], in_=fcW[:])
        icsb = cp.tile([HID, G], F32)
        nc.sync.dma_start(out=icsb[:], in_=invc[:])
        gssb = cp.tile([P, CH], F32)
        nc.sync.dma_start(out=gssb[:], in_=gslot[:])
        idsb = cp.tile([P, P], BF16)
        make_identity(nc, idsb[:])
        out1 = o1p.tile([P, CH * P], BF16)
        adloc1 = cp.tile([P, CH, 2], BF16)  # per-chunk dst attention terms
        adloc2 = cp.tile([P, CH, 1], BF16)
        asloc1 = cp.tile([P, CH, 2], BF16)  # per-chunk src terms (self-loops)
        asloc2 = cp.tile([P, CH, 1], BF16)

        # ---------------- dense 1: rows of table1 ----------------
        shard1_w = []

        def dense1_tile(t):
            nv = min(P, NPC - t * P)
            xt = dio.tile([P, P], BF16, tag="xt")
            nc.sync.dma_start(out=xt[:], in_=xT[:, t * P:(t + 1) * P])
            ps = dps.tile([P, 136], F32, tag="dtmp")
            nc.tensor.matmul(out=ps[:], lhsT=xt[:], rhs=r1sb[:], start=True, stop=True)
            row = dio.tile([P, ROW1], FP8, tag="row1")
            nc.scalar.copy(out=row[:, 0:130], in_=ps[:, 0:130])
            nc.vector.memset(row[:, 130:ROW1], 0.0)
            nc.vector.memset(row[:, 64:130:65].unsqueeze(2), 1.0)
            # small bf16 copies go to the (idle) vector engine: the scalar
            # queue is the dense1 critical path
            nc.vector.tensor_copy(out=row[:].bitcast(BF16)[:, 66:68], in_=ps[:, 130:132])
            nc.vector.tensor_copy(out=adloc1[:, t, :], in_=ps[:, 132:134])
            nc.vector.tensor_copy(out=asloc1[:, t, :], in_=ps[:, 130:132])
            # scalar queue: by queue position the row is already written, so
            # this write never head-of-line blocks; sync stays pure prefetch
            shard1_w.append(nc.scalar.dma_start(out=shard1[t * P:t * P + nv, :], in_=row[:nv, :]))

        for t in range(CH):
            dense1_tile(t)
        cc1 = nc.gpsimd.collective_compute(
            "AllGather", ALU.bypass, replica_groups=[R],
            ins=[shard1[0:NPC, :]], outs=[table1[0:N, :]])
        for w in shard1_w:
            add_dep_helper(cc1.ins, w.ins, sync=True, reason="shard1 before AG")
        shard1_w.clear()

        # ---------------- edge phase (both layers) ----------------
        # Software-pipelined: stage A (DMAs, gathers, ad one-hot matmuls) for
        # chunk c+1 is EMITTED before stage B (coefficient chain + selection
        # matmuls + fin) of chunk c, so the in-order PE queue runs c+1's ad
        # matmuls while c's selection matmuls still wait on c's gathers.
        def edge_layer(lay, gl, gh, s0t, s0e, tabA, tabB, adloc, row_w, nheads,
                       finalize, dt, as_of, shard, asloc):
            TL, TH = lay["TL"], lay["TH"]
            LO_OFF, HI_OFF, S_OFF = lay["LO_OFF"], lay["HI_OFF"], lay["S_OFF"]
            W = 65 * nheads
            # self-loop coefficients for ALL chunks in one batched op set
            cs0 = cp.tile([P, CH, nheads], BF16, tag=f"cs0{nheads}")
            nc.vector.tensor_tensor(out=cs0[:], in0=asloc[:], in1=adloc[:], op=ALU.add)
            cs1 = cp.tile([P, CH, nheads], BF16, tag=f"cs1{nheads}")
            nc.vector.scalar_tensor_tensor(out=cs1[:], in0=cs0[:], scalar=0.2,
                                           in1=cs0[:], op0=ALU.mult, op1=ALU.max)
            csa = cp.tile([P, CH, nheads], BF16, tag=f"csa{nheads}")
            nc.scalar.activation(out=csa[:], in_=cs1[:], func=AF.Exp, scale=1.0)

            def stage_a(c):
                TLc, THc = TL[c], TH[c]
                Tc = TLc + THc
                nlo, nhi = TLc * P, THc * P
                glt = ixp.tile([P, nlo // 16], I16, tag="glt")
                nc.sync.dma_start(out=glt[:], in_=gl[:, LO_OFF[c] // 16:(LO_OFF[c] + nlo) // 16])
                ght = ixp.tile([P, nhi // 16], I16, tag="ght")
                nc.sync.dma_start(out=ght[:], in_=gh[:, HI_OFF[c] // 16:(HI_OFF[c] + nhi) // 16])
                stt = stp.tile([P, Tc * P], FP8, tag="stt")
                nc.scalar.dma_start(out=stt[:], in_=s0t[:, S_OFF[c]:S_OFF[c] + Tc * P])
                set_ = stp.tile([P, Tc * P], FP8, tag="set")
                nc.scalar.dma_start(out=set_[:], in_=s0e[:, S_OFF[c]:S_OFF[c] + Tc * P])
                # each section's gather is split in half across two queues so
                # all 4 SWDGE queues drain concurrently
                hgl = gp.tile([P, TLc, row_w], dt, tag="hgl")
                hgh = gp.tile([P, THc, row_w], dt, tag="hgh")
                for buf, idx, n in ((hgl, glt, nlo), (hgh, ght, nhi)):
                    h1 = (n // 2) & ~127 or n
                    for a, b in ((0, h1), (h1, n)):
                        if a == b:
                            continue
                        paced_gather(
                            xp, out_ap=buf[:, a // P:b // P, :],
                            in_ap=(tabA if buf is hgl else tabB),
                            idxs_ap=idx[:, a // 16:b // 16],
                            num_idxs=b - a, num_idxs_reg=b - a, elem_size=row_w)
                # per-edge ad via fp8 one-hot matmuls
                adps = aps.tile([P, Tc, nheads], F32, tag="adps")
                for t in range(Tc):
                    nc.tensor.matmul(
                        out=adps[:, t, :], lhsT=stt[:, t * P:(t + 1) * P],
                        rhs=adloc[:, c, :], start=True, stop=True)
                adsb = xp.tile([P, Tc, nheads], BF16, tag="adsb")
                nc.scalar.copy(out=adsb[:], in_=adps[:])
                # own-chunk table rows -> self-loop contribution rows
                nv = min(P, NPC - c * P)
                selfr = gp.tile([P, row_w], dt, tag="selfr")
                if nv < P:
                    nc.vector.memset(selfr[:], 0.0)
                nc.sync.dma_start(out=selfr[:nv, :], in_=shard[c * P:c * P + nv, :])
                srs = gp.tile([P, W], BF16, tag="srs")
                nc.vector.tensor_tensor(
                    out=srs[:].rearrange("p (h f) -> p h f", h=nheads),
                    in0=selfr[:, 0:W].rearrange("p (h f) -> p h f", h=nheads),
                    in1=csa[:, c, :].unsqueeze(2).to_broadcast([P, nheads, 65]),
                    op=ALU.mult)
                return c, set_, hgl, hgh, adsb, srs

            def stage_b(st):
                c, set_, hgl, hgh, adsb, srs = st
                TLc, THc = TL[c], TH[c]
                Tc = TLc + THc
                secs = ((0, hgl, 0, TLc), (1, hgh, TLc, THc))
                # s = as + ad ; x = exp(max(s, 0.2 s))
                ssb = xp.tile([P, Tc, nheads], BF16, tag="ssb")
                for sec, hg_t, t0, nt in secs:
                    nc.vector.tensor_tensor(
                        out=ssb[:, t0:t0 + nt, :], in0=as_of(hg_t, 0, nt),
                        in1=adsb[:, t0:t0 + nt, :], op=ALU.add)
                s2 = xp.tile([P, Tc, nheads], BF16, tag="s2")
                nc.vector.scalar_tensor_tensor(out=s2[:], in0=ssb[:], scalar=0.2,
                                               in1=ssb[:], op0=ALU.mult, op1=ALU.max)
                xsb = xp.tile([P, Tc, nheads], BF16, tag="xsb")
                nc.scalar.activation(out=xsb[:], in_=s2[:], func=AF.Exp, scale=1.0)
                # coefficient scaling, one batched op per section:
                # rs[e, t, h*65+f] = hg[e, t, h*65+f] * xsb[e, t, h]
                rsec = []
                for sec, hg_t, t0, nt in secs:
                    rs = sxp.tile([P, nt, W], BF16, tag=f"rs{sec}")
                    nc.vector.tensor_tensor(
                        out=rs[:].rearrange("p t (h f) -> p t h f", h=nheads),
                        in0=hg_t[:, 0:nt, 0:W]
                            .rearrange("p t (h f) -> p t h f", h=nheads),
                        in1=xsb[:, t0:t0 + nt, :]
                            .unsqueeze(3).to_broadcast([P, nt, nheads, 65]),
                        op=ALU.mult)
                    rsec.append(rs)
                # selection matmuls (heads merged into one 65*nheads rhs)
                Um = ups.tile([P, W], F32, tag="Um")
                for t in range(Tc):
                    if t < TLc:
                        rs, tt = rsec[0], t
                    else:
                        rs, tt = rsec[1], t - TLc
                    s0sl = set_[:, t * P:(t + 1) * P]
                    nc.tensor.matmul(
                        out=Um[:], lhsT=s0sl, rhs=rs[:, tt, :],
                        start=(t == 0), stop=(t == Tc - 1))
                finalize(c, Um, srs)

            pend = None
            for c in range(CH):
                st = stage_a(c)
                if pend is not None:
                    stage_b(pend)
                pend = st
            stage_b(pend)

        def fin1(c, Um, srs):
            # self-loop contribution: Um += exp(leaky(as+ad)) * own row
            Umf = fin.tile([P, 130], F32, tag="Umf1")
            nc.vector.tensor_tensor(out=Umf[:], in0=Um[:], in1=srs[:], op=ALU.add)
            # clamp pad-node zero denominators so Relu(scale*0) stays 0, not NaN
            ds = fin.tile([P, 2], F32, tag="ds1")
            nc.vector.tensor_scalar(
                out=ds[:], in0=Umf[:].rearrange("p (h k) -> p h k", h=2)[:, :, 64],
                scalar1=1e-30, scalar2=None, op0=ALU.max)
            rd = fin.tile([P, 2], F32, tag="rd1")
            nc.vector.reciprocal(out=rd[:], in_=ds[:])
            for h in range(2):
                nc.scalar.activation(
                    out=out1[:, c * P + h * 64:c * P + (h + 1) * 64],
                    in_=Umf[:, h * 65:h * 65 + 64], func=AF.Relu,
                    scale=rd[:, h:h + 1])
            # fused dense-2 for this chunk
            nv = min(P, NPC - c * P)
            tp = dps.tile([P, P], BF16, tag="dtmp")
            nc.tensor.transpose(out=tp[:], in_=out1[:, c * P:(c + 1) * P], identity=idsb[:])
            h1T = dio.tile([P, P], BF16, tag="h1T")
            nc.scalar.copy(out=h1T[:], in_=tp[:])
            ps = dps.tile([P, 68], F32, tag="dtmp")
            nc.tensor.matmul(out=ps[:], lhsT=h1T[:], rhs=r2sb[:], start=True, stop=True)
            row = dio.tile([P, ROW2], BF16, tag="row2")
            nc.scalar.copy(out=row[:, 0:66], in_=ps[:, 0:66])
            nc.vector.memset(row[:, 64:65], 1.0)
            nc.vector.memset(row[:, 66:ROW2], 0.0)
            nc.scalar.copy(out=adloc2[:, c, :], in_=ps[:, 66:67])
            nc.scalar.copy(out=asloc2[:, c, :], in_=ps[:, 65:66])
            # scalar queue, NOT sync: a sync-queue write would head-of-line
            # block the next groups' glt/ght index prefetch DMAs behind this
            # group's whole compute chain.
            shard2_w.append(nc.scalar.dma_start(out=shard2[c * P:c * P + nv, :], in_=row[:nv, :]))
            if c == CH_A - 1:
                # lo-half (65%) of table2's AllGather overlaps the rest of
                # L1's edge phase; only the 35% hi-half gates L2's start
                cc2a = nc.gpsimd.collective_compute(
                    "AllGather", ALU.bypass, replica_groups=[R],
                    ins=[shard2[0:NVA, :]], outs=[table2[0:BHALF, :]])
                for w in shard2_w:
                    add_dep_helper(cc2a.ins, w.ins, sync=True, reason="shard2a before AG")

        shard2_w = []
        edge_layer(cfg.lay1, gl1, gh1, s0t1, s0e1,
                   table1[0:HALF, :], table1[HALF:N, :], adloc1, ROW1, 2, fin1,
                   FP8, lambda hg_t, a, b: hg_t[:].bitcast(BF16)[:, a:b, 66:68],
                   shard1, asloc1)

        cc2b = nc.gpsimd.collective_compute(
            "AllGather", ALU.bypass, replica_groups=[R],
            ins=[shard2[NVA:NPC, :]], outs=[table2[BHALF:N, :]])
        for w in shard2_w[CH_A:]:
            add_dep_helper(cc2b.ins, w.ins, sync=True, reason="shard2b before AG")

        # ---------------- edge layer 2 + pooling ----------------
        plT = pps.tile([HID, G], F32, name="plT")

        def fin2(c, Um, srs):
            Umf = fin.tile([P, 65], F32, tag="Umf2")
            nc.vector.tensor_tensor(out=Umf[:], in0=Um[:, 0:65],
                                    in1=srs[:], op=ALU.add)
            ds = fin.tile([P, 1], F32, tag="ds2")
            nc.vector.tensor_scalar(out=ds[:], in0=Umf[:, 64:65],
                                    scalar1=1e-30, scalar2=None, op0=ALU.max)
            rd = fin.tile([P, 1], F32, tag="rd2")
            nc.vector.reciprocal(out=rd[:], in_=ds[:])
            o2 = fin.tile([P, HID], BF16, tag="o2")
            nc.scalar.activation(out=o2[:], in_=Umf[:, 0:64], func=AF.Relu,
                                 scale=rd[:])
            sg = fin.tile([P, G], BF16, tag="sg")
            nc.vector.tensor_scalar(out=sg[:], in0=io512[:],
                                    scalar1=gssb[:, c:c + 1], scalar2=None,
                                    op0=ALU.is_equal)
            nc.tensor.matmul(out=plT[:], lhsT=o2[:], rhs=sg[:],
                             start=(c == 0), stop=(c == CH - 1))

        edge_layer(cfg.lay2, gl2, gh2, s0t2, s0e2,
                   table2[0:BHALF, :], table2[BHALF:N, :], adloc2, ROW2, 1, fin2,
                   BF16, lambda hg_t, a, b: hg_t[:, a:b, 65:66], shard2, asloc2)

        # fold invc + fcW locally, AllReduce tiny partial logits [G, 2]
        plsb = fin.tile([HID, G], F32)
        nc.vector.tensor_copy(out=plsb[:], in_=plT[:])
        nc.vector.tensor_tensor(out=plsb[:], in0=plsb[:], in1=icsb[:], op=ALU.mult)
        NB = G // P
        lgp = dps.tile([P, NB, 2], F32, tag="dtmp")
        for gt in range(NB):
            nc.tensor.matmul(out=lgp[:, gt, :], lhsT=plsb[:, gt * P:(gt + 1) * P],
                             rhs=fcsb[:], start=True, stop=True)
        lgs = fin.tile([P, NB, 2], F32, tag="lgs")
        nc.scalar.copy(out=lgs[:], in_=lgp[:])
        plw = nc.sync.dma_start(
            out=pool_loc[:].rearrange("(b p) k -> p b k", p=P), in_=lgs[:])
        ccp = nc.gpsimd.collective_compute(
            "AllReduce", ALU.add, replica_groups=[R],
            ins=[pool_loc[:]], outs=[pool_sh[:]])
        add_dep_helper(ccp.ins, plw.ins, sync=True, reason="pool write before AR")
        plr = fin.tile([P, NB, 2], F32)
        plrd = nc.sync.dma_start(
            out=plr[:], in_=pool_sh[:].rearrange("(b p) k -> p b k", p=P))
        add_dep_helper(plrd.ins, ccp.ins, sync=True, reason="AR before pool read")
        # batched log-softmax over all NB graph blocks at once
        mx = fin.tile([P, NB, 1], F32, tag="mx")
        nc.vector.tensor_reduce(out=mx[:], in_=plr[:], op=ALU.max,
                                axis=mybir.AxisListType.X)
        t1 = fin.tile([P, NB, 2], F32, tag="t1")
        nc.vector.tensor_tensor(out=t1[:], in0=plr[:],
                                in1=mx[:].to_broadcast([P, NB, 2]),
                                op=ALU.subtract)
        ex = fin.tile([P, NB, 2], F32, tag="ex")
        nc.scalar.activation(out=ex[:], in_=t1[:], func=AF.Exp)
        es = fin.tile([P, NB, 1], F32, tag="es")
        nc.vector.tensor_reduce(out=es[:], in_=ex[:], op=ALU.add,
                                axis=mybir.AxisListType.X)
        ln = fin.tile([P, NB, 1], F32, tag="ln")
        nc.scalar.activation(out=ln[:], in_=es[:], func=AF.Ln)
        lsm = fin.tile([P, NB, 2], F32, tag="lsm")
        nc.vector.tensor_tensor(out=lsm[:], in0=t1[:],
                                in1=ln[:].to_broadcast([P, NB, 2]),
                                op=ALU.subtract)
        nc.sync.dma_start(out=out_lg[:].rearrange("(b p) k -> p b k", p=P),
                          in_=lsm[:])

    nc.compile()
    return nc


# ------------------------------------------------------------------ entry

LAST_EXEC_NS = None

def kernel(x, edge_index, batch, W1, a_src1, a_dst1, b1, W2, a_src2, a_dst2, b2,
           fcW, fcb):
    x = np.asarray(x, np.float32)
    edge_index = np.asarray(edge_index, np.int64)
    batch = np.asarray(batch, np.int64)
    for b in (b1, b2, fcb):
        assert np.abs(np.asarray(b)).max() == 0.0, "nonzero bias unsupported"
    cfg = plan_cfg(N=x.shape[0], E0=edge_index.shape[1], G=512)
    in_maps = prep_inputs(cfg, x, edge_index, batch,
                          np.asarray(W1, np.float32), np.asarray(a_src1, np.float32),
                          np.asarray(a_dst1, np.float32), np.asarray(W2, np.float32),
                          np.asarray(a_src2, np.float32), np.asarray(a_dst2, np.float32),
                          np.asarray(fcW, np.float32))
    nc = build_nc(cfg)
    trace = os.environ.get("KERNEL_TRACE") == "1"
    res = run_bass_kernel_spmd(nc, in_maps, list(range(cfg.CORES)), trace=trace)
    global LAST_EXEC_NS
    LAST_EXEC_NS = res.exec_time_ns
    if trace:
        print(f"HW exec time: {res.exec_time_ns} ns "
              f"(mean {res.mean_exec_time_ns} ns)")
    return np.asarray(res.results[0]["out_lg"], np.float32)



# revision 98
# speedup vs baseline: 1.0816x; 1.0816x over previous
"""GAT classifier on 8 trn2 NeuronCores (Bass/Tile) — v2.

Sharding: 1D node partition (6250 nodes/core); edges assigned to the core
owning their dst node, sorted by dst into 128-node chunks.

v2 design (vs v1): the per-edge SWDGE gather count is halved and payloads
move to bf16. Per edge per layer there is exactly ONE gathered element:
  L1: 512B bf16 row [h0(64)|1|h1(64)|1|as0|as1|pad]
  L2: 256B bf16 row [h(64)|1|as|pad]
The dst-side attention term ad_e is produced without any gather: a host
-streamed fp8 one-hot S0T[j,e] (slot-of-edge) is matmul'd against the
local per-chunk ad table (PE, 2 cols), giving per-edge ad in PSUM.
Coefficients are computed directly: coef = exp(max(s, 0.2*s)), s = as+ad.
The segmented softmax + aggregation stays as PSUM-accumulated selection
matmuls with bf16 S matrices; denominators ride the baked-in "1" columns.
Layer tables are AllGathered between layers; pooling uses an AllReduce.
"""
import math
import os
import sys
from contextlib import ExitStack
from dataclasses import dataclass

import numpy as np

for _p in ("/opt/trn_rl_repo", "/root/.axon_site/_ro/trn_rl_repo"):
    if os.path.isdir(_p) and _p not in sys.path:
        sys.path.insert(0, _p)

import concourse.bacc as bacc
import concourse.bass as bass
import concourse.mybir as mybir
import concourse.tile as tile
from concourse.tile import add_dep_helper
from concourse.bass_utils import run_bass_kernel_spmd
from concourse.masks import make_identity

P = 128
AF = mybir.ActivationFunctionType
ALU = mybir.AluOpType
F32 = mybir.dt.float32
BF16 = mybir.dt.bfloat16
FP8 = mybir.dt.float8e4
I16 = mybir.dt.int16
NP_BF16 = mybir.dt.np(BF16)
NP_FP8 = mybir.dt.np(FP8)

ROW1 = 256   # L1 table row: fp8 slots [h0|1|h1|1|pad|as0,as1(bf16@66,67)] -> 256B
ROW2 = 128   # L2 table row: bf16 slots [h|1|as|pad] -> 256B


@dataclass
class Cfg:
    N: int = 50000
    E0: int = 800000
    IN: int = 128
    HID: int = 64
    G: int = 512
    CORES: int = 8
    NPC: int = 0
    CH: int = 0
    HALF: int = 0
    NVA: int = 0
    G_CH: int = 1
    lay1: dict = None  # layer-1 gather layout (natural, split@32768)
    lay2: dict = None  # layer-2 gather layout (block, split@NVA)

    @property
    def NCH(self):  # padded per-core node count
        return self.CH * P


def plan_cfg(N, E0, G, CORES=8):
    c = Cfg(N=N, E0=E0, G=G, CORES=CORES)
    c.G_CH = int(os.environ.get("G_CH", "1"))
    assert N % CORES == 0
    c.NPC = N // CORES
    c.CH = math.ceil(c.NPC / P)
    # Layer 1 uses the natural core-major table layout (ONE AllGather,
    # address space split at 32768 for int16 indices).  Layer 2 uses a
    # block layout (block A = every core's first NVA rows) so its table
    # AllGather can be split in two, with the A-half overlapping layer 1's
    # edge phase.
    c.HALF = 1 << 15
    assert (N - c.HALF) <= 32767
    # NVA=4096 gives layer 2's block layout the same 65/35 lo/hi split as
    # layer 1's natural layout (same padding), with block-A indices still
    # fitting int16 (max CORES*NVA-1 = 32767).
    c.NVA = 32 * P
    assert 32 * P < c.NPC and 32 <= c.CH
    assert CORES * c.NVA <= 32768 and CORES * (c.NPC - c.NVA) <= 32767
    return c


# ----------------------------------------------------------------- host prep

def prep_edges(cfg, src, dst, half_map, sfx):
    """Per-core edge arrays with per-chunk section sizes (max across cores,
    so the SPMD program shape is identical). Returns (layout, per-core dicts).
    half_map(s) -> (half, idx) arrays for the given table layout."""
    owner = dst // cfg.NPC
    per_core = []
    cnts = np.zeros((cfg.CORES, cfg.CH, 2), np.int64)
    for c in range(cfg.CORES):
        m = owner == c
        s0 = src[m]
        dl = dst[m] - c * cfg.NPC
        chunk = dl >> 7
        half, s = half_map(s0)
        order = np.lexsort((s, half, chunk))
        s, dl, chunk, half = s[order], dl[order], chunk[order], half[order]
        key = chunk * 2 + half
        cnt = np.bincount(key, minlength=cfg.CH * 2).reshape(cfg.CH, 2)
        cnts[c] = cnt
        per_core.append((s, dl, cnt))
    cmax = cnts.max(axis=0)  # [CH, 2]
    TL = np.maximum((cmax[:, 0] + 127) // P, 1)
    TH = np.maximum((cmax[:, 1] + 127) // P, 1)
    lo_sz, hi_sz = TL * P, TH * P
    lay = dict(
        TL=tuple(int(v) for v in TL), TH=tuple(int(v) for v in TH),
        LO_OFF=tuple(int(v) for v in np.concatenate([[0], np.cumsum(lo_sz)])),
        HI_OFF=tuple(int(v) for v in np.concatenate([[0], np.cumsum(hi_sz)])),
        S_OFF=tuple(int(v) for v in np.concatenate([[0], np.cumsum(lo_sz + hi_sz)])),
        TOT_LO=int(lo_sz.sum()), TOT_HI=int(hi_sz.sum()))
    lay["TOT_E"] = lay["TOT_LO"] + lay["TOT_HI"]

    out = []
    for c in range(cfg.CORES):
        s, dl, cnt = per_core[c]
        gl = np.zeros(lay["TOT_LO"], np.int16)
        gh = np.zeros(lay["TOT_HI"], np.int16)
        sl = np.full(lay["TOT_E"], 300.0, np.float32)
        ofs = np.zeros(cfg.CH * 2 + 1, np.int64)
        np.cumsum(cnt.reshape(-1), out=ofs[1:])
        for k in range(cfg.CH):
            nlo, nhi = int(cnt[k, 0]), int(cnt[k, 1])
            a = ofs[2 * k]
            gl[lay["LO_OFF"][k]:lay["LO_OFF"][k] + nlo] = s[a:a + nlo]
            sl[lay["S_OFF"][k]:lay["S_OFF"][k] + nlo] = (dl[a:a + nlo] & 127).astype(np.float32)
            b = ofs[2 * k + 1]
            gh[lay["HI_OFF"][k]:lay["HI_OFF"][k] + nhi] = s[b:b + nhi]
            shi = lay["S_OFF"][k] + lo_sz[k]
            sl[shi:shi + nhi] = (dl[b:b + nhi] & 127).astype(np.float32)

        def wrap16(a):  # idx i -> [i % 16, i // 16], replicated over 8 groups
            w = a.reshape(-1, 16).T.copy()
            return np.tile(w, (8, 1)).astype(np.int16)

        # S0T fp8 one-hot blocks: [128 j, TOT_E] — col S_OFF[c]+t*128+e is
        # one at row slot_e (pad slots 300 -> all-zero column).
        sl_t = sl.reshape(-1, P)  # [tile, e] slot values
        idx = sl_t.astype(np.int32)
        ntile = sl_t.shape[0]
        s0t = np.zeros((ntile, P, P), NP_FP8)  # [tile, e, j]
        tt, ee = np.nonzero(idx < P)
        s0t[tt, ee, idx[tt, ee]] = 1.0
        s0e = np.ascontiguousarray(s0t.transpose(1, 0, 2).reshape(P, ntile * P))
        s0t = np.ascontiguousarray(s0t.transpose(2, 0, 1).reshape(P, ntile * P))

        out.append({f"gl{sfx}": wrap16(gl), f"gh{sfx}": wrap16(gh),
                    f"s0t{sfx}": s0t, f"s0e{sfx}": s0e})
    return lay, out


def balance_perm(cfg, dst):
    """Per-core node->slot permutation equalizing per-chunk edge counts.
    Returns perm[global] -> new global id (same core, reassigned chunk)."""
    N, CORES, NPC, CH = cfg.N, cfg.CORES, cfg.NPC, cfg.CH
    deg = np.bincount(dst, minlength=N).astype(np.int64)
    perm = np.empty(N, np.int64)
    for c in range(CORES):
        d = deg[c * NPC:(c + 1) * NPC]
        order = np.argsort(-d, kind="stable")
        loads = np.zeros(CH, np.int64)
        counts = np.zeros(CH, np.int64)
        cap = np.full(CH, P, np.int64)
        cap[CH - 1] = NPC - (CH - 1) * P if NPC % P else P
        newloc = np.empty(NPC, np.int64)
        import heapq
        heap = [(0, k) for k in range(CH)]
        heapq.heapify(heap)
        for i in order:
            while True:
                l, k = heapq.heappop(heap)
                if counts[k] < cap[k]:
                    break
            newloc[i] = k * P + counts[k]
            counts[k] += 1
            loads[k] += d[i]
            if counts[k] < cap[k]:
                heapq.heappush(heap, (loads[k], k))
        perm[c * NPC:(c + 1) * NPC] = c * NPC + newloc
    return perm


def prep_inputs(cfg, x, edge_index, batch, W1, a_src1, a_dst1, W2, a_src2, a_dst2, fcW):
    # self-loops are NOT materialized as edges; their contribution is added
    # locally per chunk (coef_self * own row) in the fin callbacks.
    N, CORES, NPC, CH = cfg.N, cfg.CORES, cfg.NPC, cfg.CH
    src = edge_index[0].astype(np.int64)
    dst = edge_index[1].astype(np.int64)
    perm = balance_perm(cfg, dst)
    src, dst = perm[src], perm[dst]
    inv = np.empty(N, np.int64)
    inv[perm] = np.arange(N)
    x = x[inv]
    batch = batch[inv]

    def half_nat(s):  # layer-1: natural layout split at 32768
        half = (s >= cfg.HALF).astype(np.int64)
        return half, np.where(half == 0, s, s - cfg.HALF)

    def half_blk(s):  # layer-2: block layout split at NVA per core
        s_c, s_l = s // NPC, s % NPC
        half = (s_l >= cfg.NVA).astype(np.int64)
        return half, np.where(half == 0, s_c * cfg.NVA + s_l,
                              s_c * (NPC - cfg.NVA) + (s_l - cfg.NVA))

    cfg.lay1, edges1 = prep_edges(cfg, src, dst, half_nat, "1")
    cfg.lay2, edges2 = prep_edges(cfg, src, dst, half_blk, "2")
    edges = [e1 | e2 for e1, e2 in zip(edges1, edges2)]

    H = 2
    HID = cfg.HID
    # rhs1 cols: [W_h0(64) | 0 | W_h1(64) | 0 | as0 as1 ad0 ad1 | pad] = 136;
    # the zero gap columns become the denominator "1" slots (memset on-chip),
    # so the MM output is the table row layout directly.
    rhs1 = np.zeros((cfg.IN, 136), np.float32)
    rhs1[:, 0:64] = W1[:, 0:HID]
    rhs1[:, 65:129] = W1[:, HID:2 * HID]
    for h in range(H):
        rhs1[:, 130 + h] = W1[:, h * HID:(h + 1) * HID] @ a_src1[h]
        rhs1[:, 132 + h] = W1[:, h * HID:(h + 1) * HID] @ a_dst1[h]
    # rhs2 cols: [W2(64) | 0 | as2 ad2 | pad] = 68
    rhs2 = np.zeros((H * HID, 68), np.float32)
    rhs2[:, 0:HID] = W2
    rhs2[:, HID + 1] = W2 @ a_src2[0]
    rhs2[:, HID + 2] = W2 @ a_dst2[0]

    iota512 = np.tile(np.arange(cfg.G, dtype=np.float32), (P, 1))
    cnt = np.bincount(batch, minlength=cfg.G).astype(np.float32)
    invc = 1.0 / np.maximum(cnt, 1.0)
    invc_b = np.tile(invc, (HID, 1)).astype(np.float32)

    xT = np.zeros((cfg.IN, CORES * cfg.NCH), NP_BF16)
    gsl = np.full((CORES, cfg.NCH), 999.0, np.float32)
    for c in range(CORES):
        xT[:, c * cfg.NCH:c * cfg.NCH + NPC] = x[c * NPC:(c + 1) * NPC].T
        gsl[c, :NPC] = batch[c * NPC:(c + 1) * NPC]

    in_maps = []
    for c in range(CORES):
        in_maps.append(dict(
            xT=np.ascontiguousarray(xT[:, c * cfg.NCH:(c + 1) * cfg.NCH]),
            rhs1=rhs1.astype(NP_BF16), rhs2=rhs2.astype(NP_BF16),
            fcW=fcW.astype(np.float32),
            iota512=iota512, invc=invc_b,
            gslot=gsl[c].reshape(CH, P).T.copy(),
            **edges[c],
        ))
    return in_maps


# -------------------------------------------------------------- bass builder

def build_nc(cfg):
    N, CH, NPC = cfg.N, cfg.CH, cfg.NPC
    HID, G = cfg.HID, cfg.G
    HALF, NVA = cfg.HALF, cfg.NVA
    CH_A = NVA // P
    BHALF = cfg.CORES * NVA
    R = list(range(cfg.CORES))

    NQ = int(os.environ.get("GATHER_QUEUES", "4"))
    nc = bacc.Bacc(num_swdge_queues=NQ)
    pi = lambda n, s, d=F32: nc.declare_dram_parameter(n, s, d, isOutput=False)
    xT = pi("xT", [cfg.IN, cfg.NCH], BF16)
    rhs1 = pi("rhs1", [cfg.IN, 136], BF16)
    rhs2 = pi("rhs2", [2 * HID, 68], BF16)
    fcW = pi("fcW", [HID, 2])
    iota512 = pi("iota512", [P, G])
    invc = pi("invc", [HID, G])
    gslot = pi("gslot", [P, CH])
    gl1 = pi("gl1", [P, cfg.lay1["TOT_LO"] // 16], I16)
    gh1 = pi("gh1", [P, cfg.lay1["TOT_HI"] // 16], I16)
    s0t1 = pi("s0t1", [P, cfg.lay1["TOT_E"]], FP8)
    s0e1 = pi("s0e1", [P, cfg.lay1["TOT_E"]], FP8)
    gl2 = pi("gl2", [P, cfg.lay2["TOT_LO"] // 16], I16)
    gh2 = pi("gh2", [P, cfg.lay2["TOT_HI"] // 16], I16)
    s0t2 = pi("s0t2", [P, cfg.lay2["TOT_E"]], FP8)
    s0e2 = pi("s0e2", [P, cfg.lay2["TOT_E"]], FP8)
    out_lg = nc.declare_dram_parameter("out_lg", [G, 2], F32, isOutput=True)

    shard1 = nc.dram_tensor("shard1", [NPC, ROW1], FP8)
    table1 = nc.dram_tensor("table1", [N, ROW1], FP8, addr_space="Shared")
    shard2 = nc.dram_tensor("shard2", [NPC, ROW2], BF16)
    table2 = nc.dram_tensor("table2", [N, ROW2], BF16, addr_space="Shared")
    pool_loc = nc.dram_tensor("pool_loc", [G, 2], F32)
    pool_sh = nc.dram_tensor("pool_sh", [G, 2], F32, addr_space="Shared")

    # Gathers round-robin across SWDGE queues; each queue runs on its own Q7
    # core pair (queue q -> cores 2q,2q+1) and the ucode's own ring
    # await_space provides flow control, so no software pacing is needed.
    gather_rr = [0]

    def paced_gather(probe_pool, **kw):
        q = gather_rr[0] % NQ
        gather_rr[0] += 1
        return nc.gpsimd.dma_gather(single_packet=False, queue_num=q, **kw)

    with tile.TileContext(nc) as tc, ExitStack() as ctx:
        cp = ctx.enter_context(tc.tile_pool(name="const", bufs=1))
        dio = ctx.enter_context(tc.tile_pool(name="dio", bufs=3))
        dps = ctx.enter_context(tc.tile_pool(name="dps", bufs=2, space="PSUM"))
        o1p = ctx.enter_context(tc.tile_pool(name="o1p", bufs=1))
        ixp = ctx.enter_context(tc.tile_pool(name="ixp", bufs=10))
        gp = ctx.enter_context(tc.tile_pool(name="gp", bufs=12))
        stp = ctx.enter_context(tc.tile_pool(name="stp", bufs=6))
        sxp = ctx.enter_context(tc.tile_pool(name="sxp", bufs=6))
        xp = ctx.enter_context(tc.tile_pool(name="xp", bufs=6))
        ups = ctx.enter_context(tc.tile_pool(name="ups", bufs=3, space="PSUM"))
        aps = ctx.enter_context(tc.tile_pool(name="aps", bufs=2, space="PSUM"))
        pps = ctx.enter_context(tc.tile_pool(name="pps", bufs=1, space="PSUM"))
        fin = ctx.enter_context(tc.tile_pool(name="fin", bufs=3))

        io512 = cp.tile([P, G], F32)
        nc.sync.dma_start(out=io512[:], in_=iota512[:])
        r1sb = cp.tile([cfg.IN, 136], BF16)
        nc.sync.dma_start(out=r1sb[:], in_=rhs1[:])
        r2sb = cp.tile([2 * HID, 68], BF16)
        nc.sync.dma_start(out=r2sb[:], in_=rhs2[:])
        fcsb = cp.tile([HID, 2], F32)
        nc.sync.dma_start(out=fcsb[:], in_=fcW[:])
        icsb = cp.tile([HID, G], F32)
        nc.sync.dma_start(out=icsb[:], in_=invc[:])
        gssb = cp.tile([P, CH], F32)
        nc.sync.dma_start(out=gssb[:], in_=gslot[:])
        idsb = cp.tile([P, P], BF16)
        make_identity(nc, idsb[:])
        out1 = o1p.tile([P, CH * P], BF16)
        adloc1 = cp.tile([P, CH, 2], BF16)  # per-chunk dst attention terms
        adloc2 = cp.tile([P, CH, 1], BF16)
        asloc1 = cp.tile([P, CH, 2], BF16)  # per-chunk src terms (self-loops)
        asloc2 = cp.tile([P, CH, 1], BF16)

        # ---------------- dense 1: rows of table1 ----------------
        shard1_w = []

        def dense1_tile(t):
            nv = min(P, NPC - t * P)
            xt = dio.tile([P, P], BF16, tag="xt")
            nc.sync.dma_start(out=xt[:], in_=xT[:, t * P:(t + 1) * P])
            ps = dps.tile([P, 136], F32, tag="dtmp")
            nc.tensor.matmul(out=ps[:], lhsT=xt[:], rhs=r1sb[:], start=True, stop=True)
            row = dio.tile([P, ROW1], FP8, tag="row1")
            nc.scalar.copy(out=row[:, 0:130], in_=ps[:, 0:130])
            nc.vector.memset(row[:, 130:ROW1], 0.0)
            nc.vector.memset(row[:, 64:130:65].unsqueeze(2), 1.0)
            nc.scalar.copy(out=row[:].bitcast(BF16)[:, 66:68], in_=ps[:, 130:132])
            nc.scalar.copy(out=adloc1[:, t, :], in_=ps[:, 132:134])
            nc.scalar.copy(out=asloc1[:, t, :], in_=ps[:, 130:132])
            # scalar queue: by queue position the row is already written, so
            # this write never head-of-line blocks; sync stays pure prefetch
            shard1_w.append(nc.scalar.dma_start(out=shard1[t * P:t * P + nv, :], in_=row[:nv, :]))

        for t in range(CH):
            dense1_tile(t)
        cc1 = nc.gpsimd.collective_compute(
            "AllGather", ALU.bypass, replica_groups=[R],
            ins=[shard1[0:NPC, :]], outs=[table1[0:N, :]])
        for w in shard1_w:
            add_dep_helper(cc1.ins, w.ins, sync=True, reason="shard1 before AG")
        shard1_w.clear()

        # ---------------- edge phase (both layers) ----------------
        # Software-pipelined: stage A (DMAs, gathers, ad one-hot matmuls) for
        # chunk c+1 is EMITTED before stage B (coefficient chain + selection
        # matmuls + fin) of chunk c, so the in-order PE queue runs c+1's ad
        # matmuls while c's selection matmuls still wait on c's gathers.
        def edge_layer(lay, gl, gh, s0t, s0e, tabA, tabB, adloc, row_w, nheads,
                       finalize, dt, as_of, shard, asloc):
            TL, TH = lay["TL"], lay["TH"]
            LO_OFF, HI_OFF, S_OFF = lay["LO_OFF"], lay["HI_OFF"], lay["S_OFF"]
            W = 65 * nheads
            # self-loop coefficients for ALL chunks in one batched op set
            cs0 = cp.tile([P, CH, nheads], BF16, tag=f"cs0{nheads}")
            nc.vector.tensor_tensor(out=cs0[:], in0=asloc[:], in1=adloc[:], op=ALU.add)
            cs1 = cp.tile([P, CH, nheads], BF16, tag=f"cs1{nheads}")
            nc.vector.scalar_tensor_tensor(out=cs1[:], in0=cs0[:], scalar=0.2,
                                           in1=cs0[:], op0=ALU.mult, op1=ALU.max)
            csa = cp.tile([P, CH, nheads], BF16, tag=f"csa{nheads}")
            nc.scalar.activation(out=csa[:], in_=cs1[:], func=AF.Exp, scale=1.0)

            def stage_a(c):
                TLc, THc = TL[c], TH[c]
                Tc = TLc + THc
                nlo, nhi = TLc * P, THc * P
                glt = ixp.tile([P, nlo // 16], I16, tag="glt")
                nc.sync.dma_start(out=glt[:], in_=gl[:, LO_OFF[c] // 16:(LO_OFF[c] + nlo) // 16])
                ght = ixp.tile([P, nhi // 16], I16, tag="ght")
                nc.sync.dma_start(out=ght[:], in_=gh[:, HI_OFF[c] // 16:(HI_OFF[c] + nhi) // 16])
                stt = stp.tile([P, Tc * P], FP8, tag="stt")
                nc.scalar.dma_start(out=stt[:], in_=s0t[:, S_OFF[c]:S_OFF[c] + Tc * P])
                set_ = stp.tile([P, Tc * P], FP8, tag="set")
                nc.scalar.dma_start(out=set_[:], in_=s0e[:, S_OFF[c]:S_OFF[c] + Tc * P])
                # each section's gather is split in half across two queues so
                # all 4 SWDGE queues drain concurrently
                hgl = gp.tile([P, TLc, row_w], dt, tag="hgl")
                hgh = gp.tile([P, THc, row_w], dt, tag="hgh")
                for buf, idx, n in ((hgl, glt, nlo), (hgh, ght, nhi)):
                    h1 = (n // 2) & ~127 or n
                    for a, b in ((0, h1), (h1, n)):
                        if a == b:
                            continue
                        paced_gather(
                            xp, out_ap=buf[:, a // P:b // P, :],
                            in_ap=(tabA if buf is hgl else tabB),
                            idxs_ap=idx[:, a // 16:b // 16],
                            num_idxs=b - a, num_idxs_reg=b - a, elem_size=row_w)
                # per-edge ad via fp8 one-hot matmuls
                adps = aps.tile([P, Tc, nheads], F32, tag="adps")
                for t in range(Tc):
                    nc.tensor.matmul(
                        out=adps[:, t, :], lhsT=stt[:, t * P:(t + 1) * P],
                        rhs=adloc[:, c, :], start=True, stop=True)
                adsb = xp.tile([P, Tc, nheads], BF16, tag="adsb")
                nc.scalar.copy(out=adsb[:], in_=adps[:])
                # own-chunk table rows -> self-loop contribution rows
                nv = min(P, NPC - c * P)
                selfr = gp.tile([P, row_w], dt, tag="selfr")
                if nv < P:
                    nc.vector.memset(selfr[:], 0.0)
                nc.sync.dma_start(out=selfr[:nv, :], in_=shard[c * P:c * P + nv, :])
                srs = gp.tile([P, W], BF16, tag="srs")
                nc.vector.tensor_tensor(
                    out=srs[:].rearrange("p (h f) -> p h f", h=nheads),
                    in0=selfr[:, 0:W].rearrange("p (h f) -> p h f", h=nheads),
                    in1=csa[:, c, :].unsqueeze(2).to_broadcast([P, nheads, 65]),
                    op=ALU.mult)
                return c, set_, hgl, hgh, adsb, srs

            def stage_b(st):
                c, set_, hgl, hgh, adsb, srs = st
                TLc, THc = TL[c], TH[c]
                Tc = TLc + THc
                secs = ((0, hgl, 0, TLc), (1, hgh, TLc, THc))
                # s = as + ad ; x = exp(max(s, 0.2 s))
                ssb = xp.tile([P, Tc, nheads], BF16, tag="ssb")
                for sec, hg_t, t0, nt in secs:
                    nc.vector.tensor_tensor(
                        out=ssb[:, t0:t0 + nt, :], in0=as_of(hg_t, 0, nt),
                        in1=adsb[:, t0:t0 + nt, :], op=ALU.add)
                s2 = xp.tile([P, Tc, nheads], BF16, tag="s2")
                nc.vector.scalar_tensor_tensor(out=s2[:], in0=ssb[:], scalar=0.2,
                                               in1=ssb[:], op0=ALU.mult, op1=ALU.max)
                xsb = xp.tile([P, Tc, nheads], BF16, tag="xsb")
                nc.scalar.activation(out=xsb[:], in_=s2[:], func=AF.Exp, scale=1.0)
                # coefficient scaling, one batched op per section:
                # rs[e, t, h*65+f] = hg[e, t, h*65+f] * xsb[e, t, h]
                rsec = []
                for sec, hg_t, t0, nt in secs:
                    rs = sxp.tile([P, nt, W], BF16, tag=f"rs{sec}")
                    nc.vector.tensor_tensor(
                        out=rs[:].rearrange("p t (h f) -> p t h f", h=nheads),
                        in0=hg_t[:, 0:nt, 0:W]
                            .rearrange("p t (h f) -> p t h f", h=nheads),
                        in1=xsb[:, t0:t0 + nt, :]
                            .unsqueeze(3).to_broadcast([P, nt, nheads, 65]),
                        op=ALU.mult)
                    rsec.append(rs)
                # selection matmuls (heads merged into one 65*nheads rhs)
                Um = ups.tile([P, W], F32, tag="Um")
                for t in range(Tc):
                    if t < TLc:
                        rs, tt = rsec[0], t
                    else:
                        rs, tt = rsec[1], t - TLc
                    s0sl = set_[:, t * P:(t + 1) * P]
                    nc.tensor.matmul(
                        out=Um[:], lhsT=s0sl, rhs=rs[:, tt, :],
                        start=(t == 0), stop=(t == Tc - 1))
                finalize(c, Um, srs)

            pend = None
            for c in range(CH):
                st = stage_a(c)
                if pend is not None:
                    stage_b(pend)
                pend = st
            stage_b(pend)

        def fin1(c, Um, srs):
            # self-loop contribution: Um += exp(leaky(as+ad)) * own row
            Umf = fin.tile([P, 130], F32, tag="Umf1")
            nc.vector.tensor_tensor(out=Umf[:], in0=Um[:], in1=srs[:], op=ALU.add)
            # clamp pad-node zero denominators so Relu(scale*0) stays 0, not NaN
            ds = fin.tile([P, 2], F32, tag="ds1")
            nc.vector.tensor_scalar(
                out=ds[:], in0=Umf[:].rearrange("p (h k) -> p h k", h=2)[:, :, 64],
                scalar1=1e-30, scalar2=None, op0=ALU.max)
            rd = fin.tile([P, 2], F32, tag="rd1")
            nc.vector.reciprocal(out=rd[:], in_=ds[:])
            for h in range(2):
                nc.scalar.activation(
                    out=out1[:, c * P + h * 64:c * P + (h + 1) * 64],
                    in_=Umf[:, h * 65:h * 65 + 64], func=AF.Relu,
                    scale=rd[:, h:h + 1])
            # fused dense-2 for this chunk
            nv = min(P, NPC - c * P)
            tp = dps.tile([P, P], BF16, tag="dtmp")
            nc.tensor.transpose(out=tp[:], in_=out1[:, c * P:(c + 1) * P], identity=idsb[:])
            h1T = dio.tile([P, P], BF16, tag="h1T")
            nc.scalar.copy(out=h1T[:], in_=tp[:])
            ps = dps.tile([P, 68], F32, tag="dtmp")
            nc.tensor.matmul(out=ps[:], lhsT=h1T[:], rhs=r2sb[:], start=True, stop=True)
            row = dio.tile([P, ROW2], BF16, tag="row2")
            nc.scalar.copy(out=row[:, 0:66], in_=ps[:, 0:66])
            nc.vector.memset(row[:, 64:65], 1.0)
            nc.vector.memset(row[:, 66:ROW2], 0.0)
            nc.scalar.copy(out=adloc2[:, c, :], in_=ps[:, 66:67])
            nc.scalar.copy(out=asloc2[:, c, :], in_=ps[:, 65:66])
            # scalar queue, NOT sync: a sync-queue write would head-of-line
            # block the next groups' glt/ght index prefetch DMAs behind this
            # group's whole compute chain.
            shard2_w.append(nc.scalar.dma_start(out=shard2[c * P:c * P + nv, :], in_=row[:nv, :]))
            if c == CH_A - 1:
                # lo-half (65%) of table2's AllGather overlaps the rest of
                # L1's edge phase; only the 35% hi-half gates L2's start
                cc2a = nc.gpsimd.collective_compute(
                    "AllGather", ALU.bypass, replica_groups=[R],
                    ins=[shard2[0:NVA, :]], outs=[table2[0:BHALF, :]])
                for w in shard2_w:
                    add_dep_helper(cc2a.ins, w.ins, sync=True, reason="shard2a before AG")

        shard2_w = []
        edge_layer(cfg.lay1, gl1, gh1, s0t1, s0e1,
                   table1[0:HALF, :], table1[HALF:N, :], adloc1, ROW1, 2, fin1,
                   FP8, lambda hg_t, a, b: hg_t[:].bitcast(BF16)[:, a:b, 66:68],
                   shard1, asloc1)

        cc2b = nc.gpsimd.collective_compute(
            "AllGather", ALU.bypass, replica_groups=[R],
            ins=[shard2[NVA:NPC, :]], outs=[table2[BHALF:N, :]])
        for w in shard2_w[CH_A:]:
            add_dep_helper(cc2b.ins, w.ins, sync=True, reason="shard2b before AG")

        # ---------------- edge layer 2 + pooling ----------------
        plT = pps.tile([HID, G], F32, name="plT")

        def fin2(c, Um, srs):
            Umf = fin.tile([P, 65], F32, tag="Umf2")
            nc.vector.tensor_tensor(out=Umf[:], in0=Um[:, 0:65],
                                    in1=srs[:], op=ALU.add)
            ds = fin.tile([P, 1], F32, tag="ds2")
            nc.vector.tensor_scalar(out=ds[:], in0=Umf[:, 64:65],
                                    scalar1=1e-30, scalar2=None, op0=ALU.max)
            rd = fin.tile([P, 1], F32, tag="rd2")
            nc.vector.reciprocal(out=rd[:], in_=ds[:])
            o2 = fin.tile([P, HID], BF16, tag="o2")
            nc.scalar.activation(out=o2[:], in_=Umf[:, 0:64], func=AF.Relu,
                                 scale=rd[:])
            sg = fin.tile([P, G], BF16, tag="sg")
            nc.vector.tensor_scalar(out=sg[:], in0=io512[:],
                                    scalar1=gssb[:, c:c + 1], scalar2=None,
                                    op0=ALU.is_equal)
            nc.tensor.matmul(out=plT[:], lhsT=o2[:], rhs=sg[:],
                             start=(c == 0), stop=(c == CH - 1))

        edge_layer(cfg.lay2, gl2, gh2, s0t2, s0e2,
                   table2[0:BHALF, :], table2[BHALF:N, :], adloc2, ROW2, 1, fin2,
                   BF16, lambda hg_t, a, b: hg_t[:, a:b, 65:66], shard2, asloc2)

        # fold invc + fcW locally, AllReduce tiny partial logits [G, 2]
        plsb = fin.tile([HID, G], F32)
        nc.vector.tensor_copy(out=plsb[:], in_=plT[:])
        nc.vector.tensor_tensor(out=plsb[:], in0=plsb[:], in1=icsb[:], op=ALU.mult)
        NB = G // P
        lgp = dps.tile([P, NB, 2], F32, tag="dtmp")
        for gt in range(NB):
            nc.tensor.matmul(out=lgp[:, gt, :], lhsT=plsb[:, gt * P:(gt + 1) * P],
                             rhs=fcsb[:], start=True, stop=True)
        lgs = fin.tile([P, NB, 2], F32, tag="lgs")
        nc.scalar.copy(out=lgs[:], in_=lgp[:])
        plw = nc.sync.dma_start(
            out=pool_loc[:].rearrange("(b p) k -> p b k", p=P), in_=lgs[:])
        ccp = nc.gpsimd.collective_compute(
            "AllReduce", ALU.add, replica_groups=[R],
            ins=[pool_loc[:]], outs=[pool_sh[:]])
        add_dep_helper(ccp.ins, plw.ins, sync=True, reason="pool write before AR")
        plr = fin.tile([P, NB, 2], F32)
        plrd = nc.sync.dma_start(
            out=plr[:], in_=pool_sh[:].rearrange("(b p) k -> p b k", p=P))
        add_dep_helper(plrd.ins, ccp.ins, sync=True, reason="AR before pool read")
        # batched log-softmax over all NB graph blocks at once
        mx = fin.tile([P, NB, 1], F32, tag="mx")
        nc.vector.tensor_reduce(out=mx[:], in_=plr[:], op=ALU.max,
                                axis=mybir.AxisListType.X)
        t1 = fin.tile([P, NB, 2], F32, tag="t1")
        nc.vector.tensor_tensor(out=t1[:], in0=plr[:],
                                in1=mx[:].to_broadcast([P, NB, 2]),
                                op=ALU.subtract)
        ex = fin.tile([P, NB, 2], F32, tag="ex")
        nc.scalar.activation(out=ex[:], in_=t1[:], func=AF.Exp)
        es = fin.tile([P, NB, 1], F32, tag="es")
        nc.vector.tensor_reduce(out=es[:], in_=ex[:], op=ALU.add,
                                axis=mybir.AxisListType.X)
        ln = fin.tile([P, NB, 1], F32, tag="ln")
        nc.scalar.activation(out=ln[:], in_=es[:], func=AF.Ln)
        lsm = fin.tile([P, NB, 2], F32, tag="lsm")
        nc.vector.tensor_tensor(out=lsm[:], in0=t1[:],
                                in1=ln[:].to_broadcast([P, NB, 2]),
                                op=ALU.subtract)
        nc.sync.dma_start(out=out_lg[:].rearrange("(b p) k -> p b k", p=P),
                          in_=lsm[:])

    nc.compile()
    return nc


# ------------------------------------------------------------------ entry

LAST_EXEC_NS = None

def kernel(x, edge_index, batch, W1, a_src1, a_dst1, b1, W2, a_src2, a_dst2, b2,
           fcW, fcb):
    x = np.asarray(x, np.float32)
    edge_index = np.asarray(edge_index, np.int64)
    batch = np.asarray(batch, np.int64)
    for b in (b1, b2, fcb):
        assert np.abs(np.asarray(b)).max() == 0.0, "nonzero bias unsupported"
    cfg = plan_cfg(N=x.shape[0], E0=edge_index.shape[1], G=512)
    in_maps = prep_inputs(cfg, x, edge_index, batch,
                          np.asarray(W1, np.float32), np.asarray(a_src1, np.float32),
                          np.asarray(a_dst1, np.float32), np.asarray(W2, np.float32),
                          np.asarray(a_src2, np.float32), np.asarray(a_dst2, np.float32),
                          np.asarray(fcW, np.float32))
    nc = build_nc(cfg)
    trace = os.environ.get("KERNEL_TRACE") == "1"
    res = run_bass_kernel_spmd(nc, in_maps, list(range(cfg.CORES)), trace=trace)
    global LAST_EXEC_NS
    LAST_EXEC_NS = res.exec_time_ns
    if trace:
        print(f"HW exec time: {res.exec_time_ns} ns "
              f"(mean {res.mean_exec_time_ns} ns)")
    return np.asarray(res.results[0]["out_lg"], np.float32)



# revision 99
# speedup vs baseline: 1.1007x; 1.0176x over previous
"""GAT classifier on 8 trn2 NeuronCores (Bass/Tile) — v2.

Sharding: 1D node partition (6250 nodes/core); edges assigned to the core
owning their dst node, sorted by dst into 128-node chunks.

v2 design (vs v1): the per-edge SWDGE gather count is halved and payloads
move to bf16. Per edge per layer there is exactly ONE gathered element:
  L1: 512B bf16 row [h0(64)|1|h1(64)|1|as0|as1|pad]
  L2: 256B bf16 row [h(64)|1|as|pad]
The dst-side attention term ad_e is produced without any gather: a host
-streamed fp8 one-hot S0T[j,e] (slot-of-edge) is matmul'd against the
local per-chunk ad table (PE, 2 cols), giving per-edge ad in PSUM.
Coefficients are computed directly: coef = exp(max(s, 0.2*s)), s = as+ad.
The segmented softmax + aggregation stays as PSUM-accumulated selection
matmuls with bf16 S matrices; denominators ride the baked-in "1" columns.
Layer tables are AllGathered between layers; pooling uses an AllReduce.
"""
import math
import os
import sys
from contextlib import ExitStack
from dataclasses import dataclass

import numpy as np

for _p in ("/opt/trn_rl_repo", "/root/.axon_site/_ro/trn_rl_repo"):
    if os.path.isdir(_p) and _p not in sys.path:
        sys.path.insert(0, _p)

import concourse.bacc as bacc
import concourse.bass as bass
import concourse.mybir as mybir
import concourse.tile as tile
from concourse.tile import add_dep_helper
from concourse.bass_utils import run_bass_kernel_spmd
from concourse.masks import make_identity

P = 128
AF = mybir.ActivationFunctionType
ALU = mybir.AluOpType
F32 = mybir.dt.float32
BF16 = mybir.dt.bfloat16
FP8 = mybir.dt.float8e4
I16 = mybir.dt.int16
NP_BF16 = mybir.dt.np(BF16)
NP_FP8 = mybir.dt.np(FP8)

ROW1 = 256   # L1 table row: fp8 slots [h0|1|h1|1|pad|as0,as1(bf16@66,67)] -> 256B
ROW2 = 128   # L2 table row: bf16 slots [h|1|as|pad] -> 256B


@dataclass
class Cfg:
    N: int = 50000
    E0: int = 800000
    IN: int = 128
    HID: int = 64
    G: int = 512
    CORES: int = 8
    NPC: int = 0
    CH: int = 0
    HALF: int = 0
    NVA: int = 0
    G_CH: int = 1
    lay1: dict = None  # layer-1 gather layout (natural, split@32768)
    lay2: dict = None  # layer-2 gather layout (block, split@NVA)

    @property
    def NCH(self):  # padded per-core node count
        return self.CH * P


def plan_cfg(N, E0, G, CORES=8):
    c = Cfg(N=N, E0=E0, G=G, CORES=CORES)
    c.G_CH = int(os.environ.get("G_CH", "1"))
    assert N % CORES == 0
    c.NPC = N // CORES
    c.CH = math.ceil(c.NPC / P)
    # Layer 1 uses the natural core-major table layout (ONE AllGather,
    # address space split at 32768 for int16 indices).  Layer 2 uses a
    # block layout (block A = every core's first NVA rows) so its table
    # AllGather can be split in two, with the A-half overlapping layer 1's
    # edge phase.
    c.HALF = 1 << 15
    assert (N - c.HALF) <= 32767
    # NVA=4096 gives layer 2's block layout the same 65/35 lo/hi split as
    # layer 1's natural layout (same padding), with block-A indices still
    # fitting int16 (max CORES*NVA-1 = 32767).
    c.NVA = 32 * P
    assert 32 * P < c.NPC and 32 <= c.CH
    assert CORES * c.NVA <= 32768 and CORES * (c.NPC - c.NVA) <= 32767
    return c


# ----------------------------------------------------------------- host prep

def prep_edges(cfg, src, dst, half_map, sfx):
    """Per-core edge arrays with per-chunk section sizes (max across cores,
    so the SPMD program shape is identical). Returns (layout, per-core dicts).
    half_map(s) -> (half, idx) arrays for the given table layout."""
    owner = dst // cfg.NPC
    per_core = []
    cnts = np.zeros((cfg.CORES, cfg.CH, 2), np.int64)
    for c in range(cfg.CORES):
        m = owner == c
        s0 = src[m]
        dl = dst[m] - c * cfg.NPC
        chunk = dl >> 7
        half, s = half_map(s0)
        order = np.lexsort((s, half, chunk))
        s, dl, chunk, half = s[order], dl[order], chunk[order], half[order]
        key = chunk * 2 + half
        cnt = np.bincount(key, minlength=cfg.CH * 2).reshape(cfg.CH, 2)
        cnts[c] = cnt
        per_core.append((s, dl, cnt))
    cmax = cnts.max(axis=0)  # [CH, 2]
    TL = np.maximum((cmax[:, 0] + 127) // P, 1)
    TH = np.maximum((cmax[:, 1] + 127) // P, 1)
    lo_sz, hi_sz = TL * P, TH * P
    lay = dict(
        TL=tuple(int(v) for v in TL), TH=tuple(int(v) for v in TH),
        LO_OFF=tuple(int(v) for v in np.concatenate([[0], np.cumsum(lo_sz)])),
        HI_OFF=tuple(int(v) for v in np.concatenate([[0], np.cumsum(hi_sz)])),
        S_OFF=tuple(int(v) for v in np.concatenate([[0], np.cumsum(lo_sz + hi_sz)])),
        TOT_LO=int(lo_sz.sum()), TOT_HI=int(hi_sz.sum()))
    lay["TOT_E"] = lay["TOT_LO"] + lay["TOT_HI"]

    out = []
    for c in range(cfg.CORES):
        s, dl, cnt = per_core[c]
        gl = np.zeros(lay["TOT_LO"], np.int16)
        gh = np.zeros(lay["TOT_HI"], np.int16)
        sl = np.full(lay["TOT_E"], 300.0, np.float32)
        ofs = np.zeros(cfg.CH * 2 + 1, np.int64)
        np.cumsum(cnt.reshape(-1), out=ofs[1:])
        for k in range(cfg.CH):
            nlo, nhi = int(cnt[k, 0]), int(cnt[k, 1])
            a = ofs[2 * k]
            gl[lay["LO_OFF"][k]:lay["LO_OFF"][k] + nlo] = s[a:a + nlo]
            sl[lay["S_OFF"][k]:lay["S_OFF"][k] + nlo] = (dl[a:a + nlo] & 127).astype(np.float32)
            b = ofs[2 * k + 1]
            gh[lay["HI_OFF"][k]:lay["HI_OFF"][k] + nhi] = s[b:b + nhi]
            shi = lay["S_OFF"][k] + lo_sz[k]
            sl[shi:shi + nhi] = (dl[b:b + nhi] & 127).astype(np.float32)

        def wrap16(a):  # idx i -> [i % 16, i // 16], replicated over 8 groups
            w = a.reshape(-1, 16).T.copy()
            return np.tile(w, (8, 1)).astype(np.int16)

        # S0T fp8 one-hot blocks: [128 j, TOT_E] — col S_OFF[c]+t*128+e is
        # one at row slot_e (pad slots 300 -> all-zero column).
        sl_t = sl.reshape(-1, P)  # [tile, e] slot values
        idx = sl_t.astype(np.int32)
        ntile = sl_t.shape[0]
        s0t = np.zeros((ntile, P, P), NP_FP8)  # [tile, e, j]
        tt, ee = np.nonzero(idx < P)
        s0t[tt, ee, idx[tt, ee]] = 1.0
        s0e = np.ascontiguousarray(s0t.transpose(1, 0, 2).reshape(P, ntile * P))
        s0t = np.ascontiguousarray(s0t.transpose(2, 0, 1).reshape(P, ntile * P))

        out.append({f"gl{sfx}": wrap16(gl), f"gh{sfx}": wrap16(gh),
                    f"s0t{sfx}": s0t, f"s0e{sfx}": s0e})
    return lay, out


def balance_perm(cfg, dst):
    """Per-core node->slot permutation equalizing per-chunk edge counts.
    Returns perm[global] -> new global id (same core, reassigned chunk)."""
    N, CORES, NPC, CH = cfg.N, cfg.CORES, cfg.NPC, cfg.CH
    deg = np.bincount(dst, minlength=N).astype(np.int64)
    perm = np.empty(N, np.int64)
    for c in range(CORES):
        d = deg[c * NPC:(c + 1) * NPC]
        order = np.argsort(-d, kind="stable")
        loads = np.zeros(CH, np.int64)
        counts = np.zeros(CH, np.int64)
        cap = np.full(CH, P, np.int64)
        cap[CH - 1] = NPC - (CH - 1) * P if NPC % P else P
        newloc = np.empty(NPC, np.int64)
        import heapq
        heap = [(0, k) for k in range(CH)]
        heapq.heapify(heap)
        for i in order:
            while True:
                l, k = heapq.heappop(heap)
                if counts[k] < cap[k]:
                    break
            newloc[i] = k * P + counts[k]
            counts[k] += 1
            loads[k] += d[i]
            if counts[k] < cap[k]:
                heapq.heappush(heap, (loads[k], k))
        perm[c * NPC:(c + 1) * NPC] = c * NPC + newloc
    return perm


def prep_inputs(cfg, x, edge_index, batch, W1, a_src1, a_dst1, W2, a_src2, a_dst2, fcW):
    # self-loops are NOT materialized as edges; their contribution is added
    # locally per chunk (coef_self * own row) in the fin callbacks.
    N, CORES, NPC, CH = cfg.N, cfg.CORES, cfg.NPC, cfg.CH
    src = edge_index[0].astype(np.int64)
    dst = edge_index[1].astype(np.int64)
    perm = balance_perm(cfg, dst)
    src, dst = perm[src], perm[dst]
    inv = np.empty(N, np.int64)
    inv[perm] = np.arange(N)
    x = x[inv]
    batch = batch[inv]

    def half_nat(s):  # layer-1: natural layout split at 32768
        half = (s >= cfg.HALF).astype(np.int64)
        return half, np.where(half == 0, s, s - cfg.HALF)

    def half_blk(s):  # layer-2: block layout split at NVA per core
        s_c, s_l = s // NPC, s % NPC
        half = (s_l >= cfg.NVA).astype(np.int64)
        return half, np.where(half == 0, s_c * cfg.NVA + s_l,
                              s_c * (NPC - cfg.NVA) + (s_l - cfg.NVA))

    cfg.lay1, edges1 = prep_edges(cfg, src, dst, half_nat, "1")
    cfg.lay2, edges2 = prep_edges(cfg, src, dst, half_blk, "2")
    edges = [e1 | e2 for e1, e2 in zip(edges1, edges2)]

    H = 2
    HID = cfg.HID
    # rhs1 cols: [W_h0(64) | 0 | W_h1(64) | 0 | as0 as1 ad0 ad1 | pad] = 136;
    # the zero gap columns become the denominator "1" slots (memset on-chip),
    # so the MM output is the table row layout directly.
    rhs1 = np.zeros((cfg.IN, 136), np.float32)
    rhs1[:, 0:64] = W1[:, 0:HID]
    rhs1[:, 65:129] = W1[:, HID:2 * HID]
    for h in range(H):
        rhs1[:, 130 + h] = W1[:, h * HID:(h + 1) * HID] @ a_src1[h]
        rhs1[:, 132 + h] = W1[:, h * HID:(h + 1) * HID] @ a_dst1[h]
    # rhs2 cols: [W2(64) | 0 | as2 ad2 | pad] = 68
    rhs2 = np.zeros((H * HID, 68), np.float32)
    rhs2[:, 0:HID] = W2
    rhs2[:, HID + 1] = W2 @ a_src2[0]
    rhs2[:, HID + 2] = W2 @ a_dst2[0]

    iota512 = np.tile(np.arange(cfg.G, dtype=np.float32), (P, 1))
    cnt = np.bincount(batch, minlength=cfg.G).astype(np.float32)
    invc = 1.0 / np.maximum(cnt, 1.0)
    invc_b = np.tile(invc, (HID, 1)).astype(np.float32)

    xT = np.zeros((cfg.IN, CORES * cfg.NCH), NP_BF16)
    gsl = np.full((CORES, cfg.NCH), 999.0, np.float32)
    for c in range(CORES):
        xT[:, c * cfg.NCH:c * cfg.NCH + NPC] = x[c * NPC:(c + 1) * NPC].T
        gsl[c, :NPC] = batch[c * NPC:(c + 1) * NPC]

    in_maps = []
    for c in range(CORES):
        in_maps.append(dict(
            xT=np.ascontiguousarray(xT[:, c * cfg.NCH:(c + 1) * cfg.NCH]),
            rhs1=rhs1.astype(NP_BF16), rhs2=rhs2.astype(NP_BF16),
            fcW=fcW.astype(np.float32),
            iota512=iota512, invc=invc_b,
            gslot=gsl[c].reshape(CH, P).T.copy(),
            **edges[c],
        ))
    return in_maps


# -------------------------------------------------------------- bass builder

def build_nc(cfg):
    N, CH, NPC = cfg.N, cfg.CH, cfg.NPC
    HID, G = cfg.HID, cfg.G
    HALF, NVA = cfg.HALF, cfg.NVA
    CH_A = NVA // P
    BHALF = cfg.CORES * NVA
    R = list(range(cfg.CORES))

    NQ = int(os.environ.get("GATHER_QUEUES", "4"))
    nc = bacc.Bacc(num_swdge_queues=NQ)
    pi = lambda n, s, d=F32: nc.declare_dram_parameter(n, s, d, isOutput=False)
    xT = pi("xT", [cfg.IN, cfg.NCH], BF16)
    rhs1 = pi("rhs1", [cfg.IN, 136], BF16)
    rhs2 = pi("rhs2", [2 * HID, 68], BF16)
    fcW = pi("fcW", [HID, 2])
    iota512 = pi("iota512", [P, G])
    invc = pi("invc", [HID, G])
    gslot = pi("gslot", [P, CH])
    gl1 = pi("gl1", [P, cfg.lay1["TOT_LO"] // 16], I16)
    gh1 = pi("gh1", [P, cfg.lay1["TOT_HI"] // 16], I16)
    s0t1 = pi("s0t1", [P, cfg.lay1["TOT_E"]], FP8)
    s0e1 = pi("s0e1", [P, cfg.lay1["TOT_E"]], FP8)
    gl2 = pi("gl2", [P, cfg.lay2["TOT_LO"] // 16], I16)
    gh2 = pi("gh2", [P, cfg.lay2["TOT_HI"] // 16], I16)
    s0t2 = pi("s0t2", [P, cfg.lay2["TOT_E"]], FP8)
    s0e2 = pi("s0e2", [P, cfg.lay2["TOT_E"]], FP8)
    out_lg = nc.declare_dram_parameter("out_lg", [G, 2], F32, isOutput=True)

    shard1 = nc.dram_tensor("shard1", [NPC, ROW1], FP8)
    table1 = nc.dram_tensor("table1", [N, ROW1], FP8, addr_space="Shared")
    shard2 = nc.dram_tensor("shard2", [NPC, ROW2], BF16)
    table2 = nc.dram_tensor("table2", [N, ROW2], BF16, addr_space="Shared")
    pool_loc = nc.dram_tensor("pool_loc", [G, 2], F32)
    pool_sh = nc.dram_tensor("pool_sh", [G, 2], F32, addr_space="Shared")

    # Gathers round-robin across SWDGE queues; each queue runs on its own Q7
    # core pair (queue q -> cores 2q,2q+1) and the ucode's own ring
    # await_space provides flow control, so no software pacing is needed.
    gather_rr = [0]

    def paced_gather(probe_pool, **kw):
        q = gather_rr[0] % NQ
        gather_rr[0] += 1
        return nc.gpsimd.dma_gather(single_packet=False, queue_num=q, **kw)

    with tile.TileContext(nc) as tc, ExitStack() as ctx:
        cp = ctx.enter_context(tc.tile_pool(name="const", bufs=1))
        dio = ctx.enter_context(tc.tile_pool(name="dio", bufs=3))
        dps = ctx.enter_context(tc.tile_pool(name="dps", bufs=2, space="PSUM"))
        o1p = ctx.enter_context(tc.tile_pool(name="o1p", bufs=1))
        ixp = ctx.enter_context(tc.tile_pool(name="ixp", bufs=10))
        gp = ctx.enter_context(tc.tile_pool(name="gp", bufs=12))
        stp = ctx.enter_context(tc.tile_pool(name="stp", bufs=6))
        sxp = ctx.enter_context(tc.tile_pool(name="sxp", bufs=6))
        xp = ctx.enter_context(tc.tile_pool(name="xp", bufs=6))
        ups = ctx.enter_context(tc.tile_pool(name="ups", bufs=3, space="PSUM"))
        aps = ctx.enter_context(tc.tile_pool(name="aps", bufs=2, space="PSUM"))
        pps = ctx.enter_context(tc.tile_pool(name="pps", bufs=1, space="PSUM"))
        fin = ctx.enter_context(tc.tile_pool(name="fin", bufs=3))

        io512 = cp.tile([P, G], F32)
        nc.sync.dma_start(out=io512[:], in_=iota512[:])
        r1sb = cp.tile([cfg.IN, 136], BF16)
        nc.sync.dma_start(out=r1sb[:], in_=rhs1[:])
        r2sb = cp.tile([2 * HID, 68], BF16)
        nc.sync.dma_start(out=r2sb[:], in_=rhs2[:])
        fcsb = cp.tile([HID, 2], F32)
        nc.sync.dma_start(out=fcsb[:], in_=fcW[:])
        icsb = cp.tile([HID, G], F32)
        nc.sync.dma_start(out=icsb[:], in_=invc[:])
        gssb = cp.tile([P, CH], F32)
        nc.sync.dma_start(out=gssb[:], in_=gslot[:])
        idsb = cp.tile([P, P], BF16)
        make_identity(nc, idsb[:])
        out1 = o1p.tile([P, CH * P], BF16)
        adloc1 = cp.tile([P, CH, 2], BF16)  # per-chunk dst attention terms
        adloc2 = cp.tile([P, CH, 1], BF16)
        asloc1 = cp.tile([P, CH, 2], BF16)  # per-chunk src terms (self-loops)
        asloc2 = cp.tile([P, CH, 1], BF16)

        # ---------------- dense 1: rows of table1 ----------------
        shard1_w = []

        def dense1_tile(t):
            nv = min(P, NPC - t * P)
            xt = dio.tile([P, P], BF16, tag="xt")
            nc.sync.dma_start(out=xt[:], in_=xT[:, t * P:(t + 1) * P])
            ps = dps.tile([P, 136], F32, tag="dtmp")
            nc.tensor.matmul(out=ps[:], lhsT=xt[:], rhs=r1sb[:], start=True, stop=True)
            row = dio.tile([P, ROW1], FP8, tag="row1")
            nc.scalar.copy(out=row[:, 0:130], in_=ps[:, 0:130])
            nc.vector.memset(row[:, 130:ROW1], 0.0)
            nc.vector.memset(row[:, 64:130:65].unsqueeze(2), 1.0)
            nc.scalar.copy(out=row[:].bitcast(BF16)[:, 66:68], in_=ps[:, 130:132])
            nc.scalar.copy(out=adloc1[:, t, :], in_=ps[:, 132:134])
            nc.scalar.copy(out=asloc1[:, t, :], in_=ps[:, 130:132])
            # scalar queue: by queue position the row is already written, so
            # this write never head-of-line blocks; sync stays pure prefetch
            shard1_w.append(nc.scalar.dma_start(out=shard1[t * P:t * P + nv, :], in_=row[:nv, :]))

        for t in range(CH):
            dense1_tile(t)
        cc1 = nc.gpsimd.collective_compute(
            "AllGather", ALU.bypass, replica_groups=[R],
            ins=[shard1[0:NPC, :]], outs=[table1[0:N, :]])
        for w in shard1_w:
            add_dep_helper(cc1.ins, w.ins, sync=True, reason="shard1 before AG")
        shard1_w.clear()

        # ---------------- edge phase (both layers) ----------------
        # Software-pipelined: stage A (DMAs, gathers, ad one-hot matmuls) for
        # chunk c+1 is EMITTED before stage B (coefficient chain + selection
        # matmuls + fin) of chunk c, so the in-order PE queue runs c+1's ad
        # matmuls while c's selection matmuls still wait on c's gathers.
        def edge_layer(lay, gl, gh, s0t, s0e, tabA, tabB, adloc, row_w, nheads,
                       finalize, dt, as_of, shard, asloc):
            TL, TH = lay["TL"], lay["TH"]
            LO_OFF, HI_OFF, S_OFF = lay["LO_OFF"], lay["HI_OFF"], lay["S_OFF"]
            W = 65 * nheads
            # self-loop coefficients for ALL chunks in one batched op set
            cs0 = cp.tile([P, CH, nheads], BF16, tag=f"cs0{nheads}")
            nc.vector.tensor_tensor(out=cs0[:], in0=asloc[:], in1=adloc[:], op=ALU.add)
            cs1 = cp.tile([P, CH, nheads], BF16, tag=f"cs1{nheads}")
            nc.vector.scalar_tensor_tensor(out=cs1[:], in0=cs0[:], scalar=0.2,
                                           in1=cs0[:], op0=ALU.mult, op1=ALU.max)
            csa = cp.tile([P, CH, nheads], BF16, tag=f"csa{nheads}")
            nc.scalar.activation(out=csa[:], in_=cs1[:], func=AF.Exp, scale=1.0)

            def stage_a(c):
                TLc, THc = TL[c], TH[c]
                Tc = TLc + THc
                nlo, nhi = TLc * P, THc * P
                glt = ixp.tile([P, nlo // 16], I16, tag="glt")
                nc.sync.dma_start(out=glt[:], in_=gl[:, LO_OFF[c] // 16:(LO_OFF[c] + nlo) // 16])
                ght = ixp.tile([P, nhi // 16], I16, tag="ght")
                nc.sync.dma_start(out=ght[:], in_=gh[:, HI_OFF[c] // 16:(HI_OFF[c] + nhi) // 16])
                stt = stp.tile([P, Tc * P], FP8, tag="stt")
                nc.scalar.dma_start(out=stt[:], in_=s0t[:, S_OFF[c]:S_OFF[c] + Tc * P])
                set_ = stp.tile([P, Tc * P], FP8, tag="set")
                nc.scalar.dma_start(out=set_[:], in_=s0e[:, S_OFF[c]:S_OFF[c] + Tc * P])
                # each section's gather is split in half across two queues so
                # all 4 SWDGE queues drain concurrently
                hgl = gp.tile([P, TLc, row_w], dt, tag="hgl")
                hgh = gp.tile([P, THc, row_w], dt, tag="hgh")
                for buf, idx, n in ((hgl, glt, nlo), (hgh, ght, nhi)):
                    h1 = (n // 2) & ~127 or n
                    for a, b in ((0, h1), (h1, n)):
                        if a == b:
                            continue
                        paced_gather(
                            xp, out_ap=buf[:, a // P:b // P, :],
                            in_ap=(tabA if buf is hgl else tabB),
                            idxs_ap=idx[:, a // 16:b // 16],
                            num_idxs=b - a, num_idxs_reg=b - a, elem_size=row_w)
                # odd bump so queue assignment rotates across chunks: lo
                # sections (~65% of edges) would otherwise always land on
                # queues 0/1 and hi on 2/3, a ~1.8x per-queue imbalance
                gather_rr[0] += 1
                # per-edge ad via fp8 one-hot matmuls
                adps = aps.tile([P, Tc, nheads], F32, tag="adps")
                for t in range(Tc):
                    nc.tensor.matmul(
                        out=adps[:, t, :], lhsT=stt[:, t * P:(t + 1) * P],
                        rhs=adloc[:, c, :], start=True, stop=True)
                adsb = xp.tile([P, Tc, nheads], BF16, tag="adsb")
                nc.scalar.copy(out=adsb[:], in_=adps[:])
                # own-chunk table rows -> self-loop contribution rows
                nv = min(P, NPC - c * P)
                selfr = gp.tile([P, row_w], dt, tag="selfr")
                if nv < P:
                    nc.vector.memset(selfr[:], 0.0)
                nc.sync.dma_start(out=selfr[:nv, :], in_=shard[c * P:c * P + nv, :])
                srs = gp.tile([P, W], BF16, tag="srs")
                nc.vector.tensor_tensor(
                    out=srs[:].rearrange("p (h f) -> p h f", h=nheads),
                    in0=selfr[:, 0:W].rearrange("p (h f) -> p h f", h=nheads),
                    in1=csa[:, c, :].unsqueeze(2).to_broadcast([P, nheads, 65]),
                    op=ALU.mult)
                return c, set_, hgl, hgh, adsb, srs

            def stage_b(st):
                c, set_, hgl, hgh, adsb, srs = st
                TLc, THc = TL[c], TH[c]
                Tc = TLc + THc
                secs = ((0, hgl, 0, TLc), (1, hgh, TLc, THc))
                # s = as + ad ; x = exp(max(s, 0.2 s))
                ssb = xp.tile([P, Tc, nheads], BF16, tag="ssb")
                for sec, hg_t, t0, nt in secs:
                    nc.vector.tensor_tensor(
                        out=ssb[:, t0:t0 + nt, :], in0=as_of(hg_t, 0, nt),
                        in1=adsb[:, t0:t0 + nt, :], op=ALU.add)
                s2 = xp.tile([P, Tc, nheads], BF16, tag="s2")
                nc.vector.scalar_tensor_tensor(out=s2[:], in0=ssb[:], scalar=0.2,
                                               in1=ssb[:], op0=ALU.mult, op1=ALU.max)
                xsb = xp.tile([P, Tc, nheads], BF16, tag="xsb")
                nc.scalar.activation(out=xsb[:], in_=s2[:], func=AF.Exp, scale=1.0)
                # coefficient scaling, one batched op per section:
                # rs[e, t, h*65+f] = hg[e, t, h*65+f] * xsb[e, t, h]
                rsec = []
                for sec, hg_t, t0, nt in secs:
                    rs = sxp.tile([P, nt, W], BF16, tag=f"rs{sec}")
                    nc.vector.tensor_tensor(
                        out=rs[:].rearrange("p t (h f) -> p t h f", h=nheads),
                        in0=hg_t[:, 0:nt, 0:W]
                            .rearrange("p t (h f) -> p t h f", h=nheads),
                        in1=xsb[:, t0:t0 + nt, :]
                            .unsqueeze(3).to_broadcast([P, nt, nheads, 65]),
                        op=ALU.mult)
                    rsec.append(rs)
                # selection matmuls (heads merged into one 65*nheads rhs)
                Um = ups.tile([P, W], F32, tag="Um")
                for t in range(Tc):
                    if t < TLc:
                        rs, tt = rsec[0], t
                    else:
                        rs, tt = rsec[1], t - TLc
                    s0sl = set_[:, t * P:(t + 1) * P]
                    nc.tensor.matmul(
                        out=Um[:], lhsT=s0sl, rhs=rs[:, tt, :],
                        start=(t == 0), stop=(t == Tc - 1))
                finalize(c, Um, srs)

            pend = None
            for c in range(CH):
                st = stage_a(c)
                if pend is not None:
                    stage_b(pend)
                pend = st
            stage_b(pend)

        def fin1(c, Um, srs):
            # self-loop contribution: Um += exp(leaky(as+ad)) * own row
            Umf = fin.tile([P, 130], F32, tag="Umf1")
            nc.vector.tensor_tensor(out=Umf[:], in0=Um[:], in1=srs[:], op=ALU.add)
            # clamp pad-node zero denominators so Relu(scale*0) stays 0, not NaN
            ds = fin.tile([P, 2], F32, tag="ds1")
            nc.vector.tensor_scalar(
                out=ds[:], in0=Umf[:].rearrange("p (h k) -> p h k", h=2)[:, :, 64],
                scalar1=1e-30, scalar2=None, op0=ALU.max)
            rd = fin.tile([P, 2], F32, tag="rd1")
            nc.vector.reciprocal(out=rd[:], in_=ds[:])
            for h in range(2):
                nc.scalar.activation(
                    out=out1[:, c * P + h * 64:c * P + (h + 1) * 64],
                    in_=Umf[:, h * 65:h * 65 + 64], func=AF.Relu,
                    scale=rd[:, h:h + 1])
            # fused dense-2 for this chunk
            nv = min(P, NPC - c * P)
            tp = dps.tile([P, P], BF16, tag="dtmp")
            nc.tensor.transpose(out=tp[:], in_=out1[:, c * P:(c + 1) * P], identity=idsb[:])
            h1T = dio.tile([P, P], BF16, tag="h1T")
            nc.scalar.copy(out=h1T[:], in_=tp[:])
            ps = dps.tile([P, 68], F32, tag="dtmp")
            nc.tensor.matmul(out=ps[:], lhsT=h1T[:], rhs=r2sb[:], start=True, stop=True)
            row = dio.tile([P, ROW2], BF16, tag="row2")
            nc.scalar.copy(out=row[:, 0:66], in_=ps[:, 0:66])
            nc.vector.memset(row[:, 64:65], 1.0)
            nc.vector.memset(row[:, 66:ROW2], 0.0)
            nc.scalar.copy(out=adloc2[:, c, :], in_=ps[:, 66:67])
            nc.scalar.copy(out=asloc2[:, c, :], in_=ps[:, 65:66])
            # scalar queue, NOT sync: a sync-queue write would head-of-line
            # block the next groups' glt/ght index prefetch DMAs behind this
            # group's whole compute chain.
            shard2_w.append(nc.scalar.dma_start(out=shard2[c * P:c * P + nv, :], in_=row[:nv, :]))
            if c == CH_A - 1:
                # lo-half (65%) of table2's AllGather overlaps the rest of
                # L1's edge phase; only the 35% hi-half gates L2's start
                cc2a = nc.gpsimd.collective_compute(
                    "AllGather", ALU.bypass, replica_groups=[R],
                    ins=[shard2[0:NVA, :]], outs=[table2[0:BHALF, :]])
                for w in shard2_w:
                    add_dep_helper(cc2a.ins, w.ins, sync=True, reason="shard2a before AG")

        shard2_w = []
        edge_layer(cfg.lay1, gl1, gh1, s0t1, s0e1,
                   table1[0:HALF, :], table1[HALF:N, :], adloc1, ROW1, 2, fin1,
                   FP8, lambda hg_t, a, b: hg_t[:].bitcast(BF16)[:, a:b, 66:68],
                   shard1, asloc1)

        cc2b = nc.gpsimd.collective_compute(
            "AllGather", ALU.bypass, replica_groups=[R],
            ins=[shard2[NVA:NPC, :]], outs=[table2[BHALF:N, :]])
        for w in shard2_w[CH_A:]:
            add_dep_helper(cc2b.ins, w.ins, sync=True, reason="shard2b before AG")

        # ---------------- edge layer 2 + pooling ----------------
        plT = pps.tile([HID, G], F32, name="plT")

        def fin2(c, Um, srs):
            Umf = fin.tile([P, 65], F32, tag="Umf2")
            nc.vector.tensor_tensor(out=Umf[:], in0=Um[:, 0:65],
                                    in1=srs[:], op=ALU.add)
            ds = fin.tile([P, 1], F32, tag="ds2")
            nc.vector.tensor_scalar(out=ds[:], in0=Umf[:, 64:65],
                                    scalar1=1e-30, scalar2=None, op0=ALU.max)
            rd = fin.tile([P, 1], F32, tag="rd2")
            nc.vector.reciprocal(out=rd[:], in_=ds[:])
            o2 = fin.tile([P, HID], BF16, tag="o2")
            nc.scalar.activation(out=o2[:], in_=Umf[:, 0:64], func=AF.Relu,
                                 scale=rd[:])
            sg = fin.tile([P, G], BF16, tag="sg")
            nc.vector.tensor_scalar(out=sg[:], in0=io512[:],
                                    scalar1=gssb[:, c:c + 1], scalar2=None,
                                    op0=ALU.is_equal)
            nc.tensor.matmul(out=plT[:], lhsT=o2[:], rhs=sg[:],
                             start=(c == 0), stop=(c == CH - 1))

        edge_layer(cfg.lay2, gl2, gh2, s0t2, s0e2,
                   table2[0:BHALF, :], table2[BHALF:N, :], adloc2, ROW2, 1, fin2,
                   BF16, lambda hg_t, a, b: hg_t[:, a:b, 65:66], shard2, asloc2)

        # fold invc + fcW locally, AllReduce tiny partial logits [G, 2]
        plsb = fin.tile([HID, G], F32)
        nc.vector.tensor_copy(out=plsb[:], in_=plT[:])
        nc.vector.tensor_tensor(out=plsb[:], in0=plsb[:], in1=icsb[:], op=ALU.mult)
        NB = G // P
        lgp = dps.tile([P, NB, 2], F32, tag="dtmp")
        for gt in range(NB):
            nc.tensor.matmul(out=lgp[:, gt, :], lhsT=plsb[:, gt * P:(gt + 1) * P],
                             rhs=fcsb[:], start=True, stop=True)
        lgs = fin.tile([P, NB, 2], F32, tag="lgs")
        nc.scalar.copy(out=lgs[:], in_=lgp[:])
        plw = nc.sync.dma_start(
            out=pool_loc[:].rearrange("(b p) k -> p b k", p=P), in_=lgs[:])
        ccp = nc.gpsimd.collective_compute(
            "AllReduce", ALU.add, replica_groups=[R],
            ins=[pool_loc[:]], outs=[pool_sh[:]])
        add_dep_helper(ccp.ins, plw.ins, sync=True, reason="pool write before AR")
        plr = fin.tile([P, NB, 2], F32)
        plrd = nc.sync.dma_start(
            out=plr[:], in_=pool_sh[:].rearrange("(b p) k -> p b k", p=P))
        add_dep_helper(plrd.ins, ccp.ins, sync=True, reason="AR before pool read")
        # batched log-softmax over all NB graph blocks at once
        mx = fin.tile([P, NB, 1], F32, tag="mx")
        nc.vector.tensor_reduce(out=mx[:], in_=plr[:], op=ALU.max,
                                axis=mybir.AxisListType.X)
        t1 = fin.tile([P, NB, 2], F32, tag="t1")
        nc.vector.tensor_tensor(out=t1[:], in0=plr[:],
                                in1=mx[:].to_broadcast([P, NB, 2]),
                                op=ALU.subtract)
        ex = fin.tile([P, NB, 2], F32, tag="ex")
        nc.scalar.activation(out=ex[:], in_=t1[:], func=AF.Exp)
        es = fin.tile([P, NB, 1], F32, tag="es")
        nc.vector.tensor_reduce(out=es[:], in_=ex[:], op=ALU.add,
                                axis=mybir.AxisListType.X)
        ln = fin.tile([P, NB, 1], F32, tag="ln")
        nc.scalar.activation(out=ln[:], in_=es[:], func=AF.Ln)
        lsm = fin.tile([P, NB, 2], F32, tag="lsm")
        nc.vector.tensor_tensor(out=lsm[:], in0=t1[:],
                                in1=ln[:].to_broadcast([P, NB, 2]),
                                op=ALU.subtract)
        nc.sync.dma_start(out=out_lg[:].rearrange("(b p) k -> p b k", p=P),
                          in_=lsm[:])

    nc.compile()
    return nc


# ------------------------------------------------------------------ entry

LAST_EXEC_NS = None

def kernel(x, edge_index, batch, W1, a_src1, a_dst1, b1, W2, a_src2, a_dst2, b2,
           fcW, fcb):
    x = np.asarray(x, np.float32)
    edge_index = np.asarray(edge_index, np.int64)
    batch = np.asarray(batch, np.int64)
    for b in (b1, b2, fcb):
        assert np.abs(np.asarray(b)).max() == 0.0, "nonzero bias unsupported"
    cfg = plan_cfg(N=x.shape[0], E0=edge_index.shape[1], G=512)
    in_maps = prep_inputs(cfg, x, edge_index, batch,
                          np.asarray(W1, np.float32), np.asarray(a_src1, np.float32),
                          np.asarray(a_dst1, np.float32), np.asarray(W2, np.float32),
                          np.asarray(a_src2, np.float32), np.asarray(a_dst2, np.float32),
                          np.asarray(fcW, np.float32))
    nc = build_nc(cfg)
    trace = os.environ.get("KERNEL_TRACE") == "1"
    res = run_bass_kernel_spmd(nc, in_maps, list(range(cfg.CORES)), trace=trace)
    global LAST_EXEC_NS
    LAST_EXEC_NS = res.exec_time_ns
    if trace:
        print(f"HW exec time: {res.exec_time_ns} ns "
              f"(mean {res.mean_exec_time_ns} ns)")
    return np.asarray(res.results[0]["out_lg"], np.float32)

